# revision 8
# baseline (speedup 1.0000x reference)
"""Trainium2 Bass kernel for the NonIsotropic vMF head (v2).

Contract: kernel(**inputs) takes FULL unsharded inputs (as produced by
setup_inputs()) and returns the FULL [S=8, B=64, C=1000] float32 output.

v2 strategy (vs v1 baseline):
  * 5 consolidated input DMAs issued in parallel from 5 engine queues
    (v1: ~24 serialized on Sync).
  * W0/W1/features transposed HOST-side; MLP runs as bf16 hi/lo split
    (3 bf16 matmuls emulate an fp32 matmul to ~1e-6 rel) -- kappa keeps
    fp32-grade accuracy for the rejection-accept margins.
  * Single activation table: only {exp, ln, relu, copy, identity, square}
    are used on the scalar engine; sqrt/rsqrt computed as exp(+-0.5*ln x).
  * Accept test reformulated reciprocal-free:
        margin >= 0  <=>  (E - 127*log1p(x) - logu) * denom >= 2ab.
  * Sample assembly + Householder reflection done directly in the
    transposed [D, S*B] domain with 5 small bf16 matmuls (block-diag
    broadcast trick) instead of 8 fp32 PE transposes + per-sample loops.
  * Class matmuls (num/den) in bf16 (output |val| >= 30, tol 2e-2).

RNG draws (beta/uniform/normal, key 42) are input-independent and
generated host-side exactly as the reference does, shipped as constants.
"""

import numpy as np

S, B, D, K, C, H = 8, 64, 128, 32, 1000, 256
NCORES = 8
CLOC = C // NCORES            # 125 classes per core
SB = S * B                    # 512
SK = S * K                    # 256
M1 = float(D - 1)             # 127.0
LN127 = float(np.log(M1))
LN2PI = float(np.log(2.0 * np.pi))

# ---- geom pack (bf16) [128, G_COLS]: host constants ----
G_ID = 0                      # ident        [0:128, 0:128]
G_VP = 128                    # Vp           [0:128, 128:640] row0=ones, 1..=vT
G_PICK = 640                  # pick         [0:16, 640:768]
G_ID8 = 768                   # ident64 x8   [0:64, 768:1280]
G_ONE = 1280                  # ones column  [0:128, 1280:1281]
G_MSK = 1281                  # blockdiag mask [0:16, 1281:1289]
G_COLS = 1289

# ---- mlp pack (bf16) [128, M_COLS]: input-derived hi/lo weights ----
M_W0H, M_W0L = 0, 256
M_W1H, M_W1L = 512, 1024      # four 128-col blocks each, order (i,j)
M_FH, M_FL = 1536, 1600
M_W2H, M_W2L = 1664, 1666
M_COLS = 1668

# ---- c32 pack (fp32) [128, C_COLS] ----
C_B0 = 0                      # [0:128, 0:2]
C_B1 = 2                      # [0:128, 2:4]
C_ID = 4                      # fp32 ident   [0:125, 4:129]
C_ONER = 129                  # ones row     [0:1, 129:257]
C_RM = 257                    # rmask        [0:64, 257:513]
C_COLS = 513

# ---- in64 pack (fp32) [64, I_COLS] ----
I_EPS, I_LOGU, I_FEAT, I_B2 = 0, 256, 512, 640
I_COLS = 641

_cache = {}


def _bf16(x):
    import ml_dtypes
    return np.ascontiguousarray(np.asarray(x, np.float32).astype(ml_dtypes.bfloat16))


def _hilo(x):
    import ml_dtypes
    x = np.asarray(x, np.float32)
    hi = x.astype(ml_dtypes.bfloat16)
    lo = (x - hi.astype(np.float32)).astype(ml_dtypes.bfloat16)
    return np.ascontiguousarray(hi), np.ascontiguousarray(lo)


def _host_constants():
    """RNG draws of the reference sampler (key 42) + geometry constants."""
    if "rng" in _cache:
        return _cache["rng"]
    import jax
    import jax.numpy as jnp
    import ml_dtypes

    cpu = jax.devices("cpu")[0]
    with jax.default_device(cpu):
        key = jax.random.key(42)
        k_eps, k_u, k_v = jax.random.split(key, 3)
        alpha = M1 / 2.0
        eps = np.asarray(jax.random.beta(k_eps, alpha, alpha, (K, S, B)), np.float32)
        u = jax.random.uniform(k_u, (K, S, B), jnp.float32, minval=1e-7, maxval=1.0)
        logu = np.asarray(jnp.log(u), np.float32)
        vraw = jax.random.normal(k_v, (S, B, D - 1), jnp.float32)
        vn = np.asarray(
            vraw / jnp.maximum(jnp.linalg.norm(vraw, axis=-1, keepdims=True), 1e-12),
            np.float32,
        )
    eps_b = np.ascontiguousarray(np.transpose(eps, (2, 1, 0)).reshape(B, SK))
    logu_b = np.ascontiguousarray(np.transpose(logu, (2, 1, 0)).reshape(B, SK))

    geom = np.zeros((128, G_COLS), np.float32)
    geom[:, G_ID:G_ID + 128] = np.eye(128)
    geom[0, G_VP:G_VP + SB] = 1.0
    geom[1:128, G_VP:G_VP + SB] = np.transpose(vn, (2, 0, 1)).reshape(D - 1, SB)
    # pick[2s, 0] = 1 (w row), pick[2s+1, 1:] = 1 (sm rows)
    geom[0:16:2, G_PICK] = 1.0
    geom[1:16:2, G_PICK + 1:G_PICK + 128] = 1.0
    geom[0:64, G_ID8:G_ID8 + SB] = np.concatenate([np.eye(64)] * S, axis=1)
    geom[:, G_ONE] = 1.0
    for q in range(16):
        geom[q, G_MSK + q // 2] = 1.0
    geom_bf = np.ascontiguousarray(geom.astype(ml_dtypes.bfloat16))

    rmask = np.ones((B, SK), np.float32)
    rmask[:, 0::K] = 0.0

    _cache["rng"] = (eps_b, logu_b, geom_bf, rmask)
    return _cache["rng"]


def build_nc():
    import concourse.bass as bass
    import concourse.mybir as mybir
    from concourse import bacc, tile

    fp = mybir.dt.float32
    bf = mybir.dt.bfloat16
    Alu = mybir.AluOpType
    Act = mybir.ActivationFunctionType

    nc = bacc.Bacc(None)

    d_mlp = nc.declare_dram_parameter("mlp", [128, M_COLS], bf, isOutput=False)
    d_geom = nc.declare_dram_parameter("geom", [128, G_COLS], bf, isOutput=False)
    d_c32 = nc.declare_dram_parameter("c32", [128, C_COLS], fp, isOutput=False)
    d_in64 = nc.declare_dram_parameter("in64", [B, I_COLS], fp, isOutput=False)
    d_wcls = nc.declare_dram_parameter("wcls", [CLOC, 2 * D], fp, isOutput=False)
    d_out = nc.declare_dram_parameter("out", [SB, CLOC], fp, isOutput=True)

    def _emit(tc):
        with (
            tc.tile_pool(name="w", bufs=1) as wp,
            tc.tile_pool(name="s", bufs=1) as sp,
            tc.tile_pool(name="scr", bufs=4) as scrp,
            tc.tile_pool(name="pp", bufs=2, space="PSUM") as pp,
        ):
            # ================= loads (parallel issue, 5 queues) =============
            mlp = wp.tile([128, M_COLS], bf)
            nc.sync.dma_start(mlp[:], d_mlp[:])
            wcl = wp.tile([CLOC, 2 * D], fp)
            nc.sync.dma_start(wcl[:], d_wcls[:])
            c32 = wp.tile([128, C_COLS], fp)
            nc.scalar.dma_start(c32[:], d_c32[:])
            in64 = wp.tile([B, I_COLS], fp)
            nc.scalar.dma_start(in64[:], d_in64[:])
            geom = wp.tile([128, G_COLS], bf)
            nc.gpsimd.dma_start(geom[:], d_geom[:])

            eps = in64[:, I_EPS:I_EPS + SK]
            logu = in64[:, I_LOGU:I_LOGU + SK]
            feat = in64[:, I_FEAT:I_FEAT + D]
            b2r = in64[:, I_B2:I_B2 + 1]
            wmu = wcl[:, 0:D]
            wk = wcl[:, D:2 * D]

            # ================= MLP (bf16 hi/lo) ============================
            fTh = mlp[:, M_FH:M_FH + B]
            fTl = mlp[:, M_FL:M_FL + B]
            h0r = [sp.tile([128, B], fp, name=f"h0r{j}") for j in range(2)]
            h0h = [sp.tile([128, B], bf, name=f"h0h{j}") for j in range(2)]
            h0l = [sp.tile([128, B], bf, name=f"h0l{j}") for j in range(2)]
            for j in range(2):
                w0h = mlp[:, M_W0H + j * 128:M_W0H + (j + 1) * 128]
                w0l = mlp[:, M_W0L + j * 128:M_W0L + (j + 1) * 128]
                pm = pp.tile([128, B], fp, tag="a")
                nc.tensor.matmul(pm[:], w0h, fTh, start=True, stop=False)
                nc.tensor.matmul(pm[:], w0h, fTl, start=False, stop=False)
                nc.tensor.matmul(pm[:], w0l, fTh, start=False, stop=True)
                nc.scalar.activation(h0r[j][:], pm[:], Act.Relu,
                                     bias=c32[:, C_B0 + j:C_B0 + j + 1], scale=1.0)
                nc.gpsimd.tensor_copy(h0h[j][:], h0r[j][:])
                nc.vector.scalar_tensor_tensor(h0l[j][:], h0h[j][:], -1.0,
                                               h0r[j][:], op0=Alu.mult, op1=Alu.add)

            # ---- uh chain part 1 (needs only feat; fills scalar idle) ----
            fsq = scrp.tile([B, D], fp, tag="scBD")
            ssf = sp.tile([B, 1], fp)
            nc.scalar.activation(fsq[:], feat, Act.Square, accum_out=ssf[:])
            lnf = scrp.tile([B, 1], fp, tag="sc")
            nc.scalar.activation(lnf[:], ssf[:], Act.Ln)
            rnf = sp.tile([B, 1], fp)
            nc.scalar.activation(rnf[:], lnf[:], Act.Exp, scale=-0.5)
            em = sp.tile([B, D], fp)
            nc.vector.tensor_scalar(em[:], feat, rnf[:], -1.0, Alu.mult, Alu.mult)
            nc.vector.tensor_scalar_add(em[:, 0:1], em[:, 0:1], 1.0)

            h1r = [sp.tile([128, B], fp, name=f"h1r{j}") for j in range(2)]
            h1h = [sp.tile([128, B], bf, name=f"h1h{j}") for j in range(2)]
            h1l = [sp.tile([128, B], bf, name=f"h1l{j}") for j in range(2)]
            for j in range(2):
                pm = pp.tile([128, B], fp, tag="a")
                first = True
                for i in range(2):
                    w1h = mlp[:, M_W1H + (i * 2 + j) * 128:M_W1H + (i * 2 + j + 1) * 128]
                    w1l = mlp[:, M_W1L + (i * 2 + j) * 128:M_W1L + (i * 2 + j + 1) * 128]
                    nc.tensor.matmul(pm[:], w1h, h0h[i][:], start=first, stop=False)
                    nc.tensor.matmul(pm[:], w1h, h0l[i][:], start=False, stop=False)
                    nc.tensor.matmul(pm[:], w1l, h0h[i][:], start=False,
                                     stop=(i == 1))
                    first = False
                nc.scalar.activation(h1r[j][:], pm[:], Act.Relu,
                                     bias=c32[:, C_B1 + j:C_B1 + j + 1], scale=1.0)
                nc.gpsimd.tensor_copy(h1h[j][:], h1r[j][:])
                nc.vector.scalar_tensor_tensor(h1l[j][:], h1h[j][:], -1.0,
                                               h1r[j][:], op0=Alu.mult, op1=Alu.add)

            # ---- uh chain part 2 ----
            esq = scrp.tile([B, D], fp, tag="scBD")
            sse = sp.tile([B, 1], fp)
            nc.scalar.activation(esq[:], em[:], Act.Square, accum_out=sse[:])
            lne = scrp.tile([B, 1], fp, tag="sc")
            nc.scalar.activation(lne[:], sse[:], Act.Ln)
            rne = sp.tile([B, 1], fp)
            nc.scalar.activation(rne[:], lne[:], Act.Exp, scale=-0.5)
            uhb = sp.tile([B, D], bf)
            nc.vector.tensor_scalar_mul(uhb[:], em[:], rne[:])

            # ---- h2 + softplus ----
            pm2 = pp.tile([B, 1], fp, tag="a")
            for j in range(2):
                w2h = mlp[:, M_W2H + j:M_W2H + j + 1]
                w2l = mlp[:, M_W2L + j:M_W2L + j + 1]
                nc.tensor.matmul(pm2[:], h1h[j][:], w2h, start=(j == 0), stop=False)
                nc.tensor.matmul(pm2[:], h1h[j][:], w2l, start=False, stop=False)
                nc.tensor.matmul(pm2[:], h1l[j][:], w2h, start=False, stop=(j == 1))
            eh2 = sp.tile([B, 1], fp)
            nc.scalar.activation(eh2[:], pm2[:], Act.Exp, bias=b2r, scale=1.0)
            kapb = sp.tile([B, 1], fp)
            nc.scalar.activation(kapb[:], eh2[:], Act.Ln, bias=1.0, scale=1.0)
            nc.vector.tensor_scalar_add(kapb[:], kapb[:], 1e-6)

            # ================= sampler scalars [B,1] =======================
            k2 = scrp.tile([B, 1], fp, tag="sc")
            nc.vector.tensor_mul(k2[:], kapb[:], kapb[:])
            nc.vector.tensor_scalar(k2[:], k2[:], 4.0, M1 * M1, Alu.mult, Alu.add)
            lnk2 = scrp.tile([B, 1], fp, tag="sc")
            nc.scalar.activation(lnk2[:], k2[:], Act.Ln)
            sq = sp.tile([B, 1], fp)
            nc.scalar.activation(sq[:], lnk2[:], Act.Exp, scale=0.5)
            b_ = sp.tile([B, 1], fp)
            nc.vector.scalar_tensor_tensor(b_[:], kapb[:], -2.0, sq[:],
                                           op0=Alu.mult, op1=Alu.add)
            nc.vector.tensor_scalar_mul(b_[:], b_[:], 1.0 / M1)
            a_ = sp.tile([B, 1], fp)
            nc.vector.scalar_tensor_tensor(a_[:], kapb[:], 2.0, sq[:],
                                           op0=Alu.mult, op1=Alu.add)
            nc.vector.tensor_scalar(a_[:], a_[:], M1, 0.25, Alu.add, Alu.mult)
            ab = sp.tile([B, 1], fp)
            nc.vector.tensor_mul(ab[:], a_[:], b_[:])
            opb = scrp.tile([B, 1], fp, tag="sc")
            nc.vector.tensor_scalar_add(opb[:], b_[:], 1.0)
            r1pb = scrp.tile([B, 1], fp, tag="sc")
            nc.vector.reciprocal(r1pb[:], opb[:])
            d_ = sp.tile([B, 1], fp)
            nc.vector.scalar_tensor_tensor(d_[:], ab[:], 4.0, r1pb[:],
                                           op0=Alu.mult, op1=Alu.mult)
            nc.vector.tensor_scalar_add(d_[:], d_[:], -M1 * LN127)
            l2ab = scrp.tile([B, 1], fp, tag="sc")
            nc.scalar.activation(l2ab[:], ab[:], Act.Ln, scale=2.0)
            E = sp.tile([B, 1], fp)
            nc.vector.scalar_tensor_tensor(E[:], l2ab[:], M1, d_[:],
                                           op0=Alu.mult, op1=Alu.add)
            p2ab = sp.tile([B, 1], fp)
            nc.vector.tensor_scalar_mul(p2ab[:], ab[:], 2.0)
            ncm = sp.tile([B, 1], fp)
            nc.vector.tensor_scalar_add(ncm[:], b_[:], -1.0)
            ncp = sp.tile([B, 1], fp)
            nc.vector.tensor_scalar(ncp[:], b_[:], -1.0, -1.0, Alu.mult, Alu.add)

            # ================= class shard stats (gpsimd + scalar) ========
            kapc = sp.tile([CLOC, D], fp)
            nc.gpsimd.tensor_scalar_max(kapc[:], wk, 0.1)
            msq = scrp.tile([CLOC, D], fp, tag="scCD")
            ssm = sp.tile([CLOC, 1], fp)
            nc.scalar.activation(msq[:], wmu, Act.Square, accum_out=ssm[:])
            lnsm = scrp.tile([CLOC, 1], fp, tag="scC")
            nc.scalar.activation(lnsm[:], ssm[:], Act.Ln)
            rnm = sp.tile([CLOC, 1], fp)
            nc.scalar.activation(rnm[:], lnsm[:], Act.Exp, scale=-0.5)
            scm = sp.tile([CLOC, D], fp)
            nc.gpsimd.tensor_mul(scm[:], wmu, kapc[:])
            nc.vector.tensor_scalar_mul(scm[:], scm[:], rnm[:])
            csq = scrp.tile([CLOC, D], fp, tag="scCD")
            ssc = sp.tile([CLOC, 1], fp)
            nc.scalar.activation(csq[:], scm[:], Act.Square, accum_out=ssc[:])
            Ppb = sp.tile([CLOC, D], bf)
            nc.gpsimd.tensor_mul(Ppb[:], kapc[:], scm[:])
            Qqb = sp.tile([CLOC, D], bf)
            nc.gpsimd.tensor_mul(Qqb[:], kapc[:], kapc[:])
            lkt = scrp.tile([CLOC, D], fp, tag="scCD")
            slk = sp.tile([CLOC, 1], fp)
            nc.scalar.activation(lkt[:], kapc[:], Act.Ln, accum_out=slk[:])
            G = sp.tile([CLOC, 1], fp)
            nc.gpsimd.tensor_scalar_add(G[:], ssc[:], 63.0 * 63.0)
            lnG = sp.tile([CLOC, 1], fp)
            nc.scalar.activation(lnG[:], G[:], Act.Ln)
            eta = sp.tile([CLOC, 1], fp)
            nc.scalar.activation(eta[:], lnG[:], Act.Exp, scale=0.5)
            etap = scrp.tile([CLOC, 1], fp, tag="scC")
            nc.gpsimd.tensor_scalar_add(etap[:], eta[:], 63.0)
            l63 = scrp.tile([CLOC, 1], fp, tag="scC")
            nc.scalar.activation(l63[:], etap[:], Act.Ln)
            lnssc = scrp.tile([CLOC, 1], fp, tag="scC")
            nc.scalar.activation(lnssc[:], ssc[:], Act.Ln)
            c1 = scrp.tile([CLOC, 1], fp, tag="scC")
            nc.gpsimd.tensor_scalar_mul(c1[:], l63[:], 63.0)
            nc.gpsimd.tensor_sub(c1[:], c1[:], eta[:])
            c2 = scrp.tile([CLOC, 1], fp, tag="scC")
            nc.gpsimd.tensor_scalar_mul(c2[:], lnssc[:], -0.5)
            nc.gpsimd.tensor_add(c2[:], c2[:], slk[:])
            nc.gpsimd.tensor_add(c1[:], c1[:], c2[:])
            cst = sp.tile([CLOC, 1], fp)
            nc.gpsimd.tensor_scalar_mul(cst[:], lnG[:], 0.25)
            nc.gpsimd.tensor_add(cst[:], cst[:], c1[:])
            nc.gpsimd.tensor_scalar_add(cst[:], cst[:], -63.5 * LN2PI)

            # class transposes + const broadcast
            identB = geom[:, G_ID:G_ID + 128]
            ps = pp.tile([128, CLOC], bf, tag="a")
            nc.tensor.transpose(ps[:], Ppb[:], identB[0:CLOC, 0:CLOC])
            PpTs = sp.tile([128, CLOC], bf)
            nc.scalar.copy(PpTs[:], ps[:])
            ps = pp.tile([128, CLOC], bf, tag="a")
            nc.tensor.transpose(ps[:], Qqb[:], identB[0:CLOC, 0:CLOC])
            QqTs = sp.tile([128, CLOC], bf)
            nc.vector.tensor_copy(QqTs[:], ps[:])
            ps = pp.tile([1, CLOC], fp, tag="a")
            nc.tensor.transpose(ps[:], cst[:], c32[0:CLOC, C_ID:C_ID + CLOC])
            cstTs = sp.tile([1, CLOC], fp)
            nc.scalar.copy(cstTs[:], ps[:])
            ps_cb = pp.tile([128, CLOC], fp, tag="cb", bufs=1)
            nc.tensor.matmul(ps_cb[:], c32[0:1, C_ONER:C_ONER + 128], cstTs[:],
                             start=True, stop=True)

            # ================= accept [B, SK], split V/G ===================
            x_ = sp.tile([B, SK], fp)
            den = sp.tile([B, SK], fp)
            u_ = sp.tile([B, SK], fp)
            acc = sp.tile([B, SK], fp)
            A = sp.tile([B, SK], fp)
            HF = SK // 2
            nc.vector.tensor_scalar_mul(x_[:], eps, ncm[:])
            nc.gpsimd.tensor_scalar_add(den[:], x_[:], 1.0)
            nc.vector.tensor_scalar(u_[:], x_[:], 1.0 / 3.0, -0.5, Alu.mult, Alu.add)
            nc.vector.tensor_mul(u_[:, 0:HF], u_[:, 0:HF], x_[:, 0:HF])
            nc.gpsimd.tensor_mul(u_[:, HF:SK], u_[:, HF:SK], x_[:, HF:SK])
            nc.vector.scalar_tensor_tensor(acc[:, 0:HF], u_[:, 0:HF], 1.0,
                                           x_[:, 0:HF], op0=Alu.add, op1=Alu.mult)
            nc.gpsimd.tensor_scalar_add(u_[:, HF:SK], u_[:, HF:SK], 1.0)
            nc.gpsimd.tensor_mul(acc[:, HF:SK], u_[:, HF:SK], x_[:, HF:SK])
            nc.vector.tensor_scalar(acc[:], acc[:], -M1, E[:], Alu.mult, Alu.add)
            nc.vector.tensor_sub(acc[:, 0:HF], acc[:, 0:HF], logu[:, 0:HF])
            nc.gpsimd.tensor_sub(acc[:, HF:SK], acc[:, HF:SK], logu[:, HF:SK])
            nc.vector.tensor_mul(acc[:, 0:HF], acc[:, 0:HF], den[:, 0:HF])
            nc.gpsimd.tensor_mul(acc[:, HF:SK], acc[:, HF:SK], den[:, HF:SK])
            nc.vector.tensor_scalar(A[:], acc[:], p2ab[:], None, Alu.is_ge)

            P = sp.tile([B, SK], fp)
            nc.vector.tensor_tensor_scan(P[:], c32[0:B, C_RM:C_RM + SK], A[:],
                                         0.0, op0=Alu.mult, op1=Alu.max)
            Pm1 = sp.tile([B, SK], fp)
            nc.vector.tensor_copy(Pm1[:, 1:SK], P[:, 0:SK - 1])
            Pm1_v = Pm1.rearrange("p (s r) -> p s r", r=K)
            nc.gpsimd.memset(Pm1_v[:, :, 0:1], 0.0)
            first = sp.tile([B, SK], fp)
            nc.vector.tensor_sub(first[:], P[:], Pm1[:])
            prod = sp.tile([B, SK], fp)
            nc.vector.tensor_mul(prod[:], eps, first[:])
            esel = sp.tile([B, S], fp)
            nc.vector.tensor_reduce(esel[:],
                                    prod.rearrange("p (s r) -> p s r", r=K),
                                    axis=mybir.AxisListType.X, op=Alu.add)
            fb = scrp.tile([B, S], fp, tag="sc8")
            nc.vector.scalar_tensor_tensor(fb[:], P[:, K - 1::K], 1.0, eps[:, 0::K],
                                           op0=Alu.subtract, op1=Alu.mult)
            nc.vector.tensor_sub(esel[:], esel[:], fb[:])

            # ================= w, sm -> interleaved pack [B, 16] ===========
            pack = sp.tile([B, 2 * S], bf)
            pk = pack.rearrange("p (s two) -> p s two", two=2)
            n1 = scrp.tile([B, S], fp, tag="sc8")
            nc.vector.tensor_scalar(n1[:], esel[:], ncp[:], 1.0, Alu.mult, Alu.add)
            d1 = scrp.tile([B, S], fp, tag="sc8")
            nc.vector.tensor_scalar(d1[:], esel[:], ncm[:], 1.0, Alu.mult, Alu.add)
            rd1 = scrp.tile([B, S], fp, tag="sc8")
            nc.vector.reciprocal(rd1[:], d1[:])
            w_ = sp.tile([B, S], fp)
            nc.vector.tensor_mul(w_[:], n1[:], rd1[:])
            nc.vector.tensor_copy(pk[:, :, 0:1], w_.rearrange("p (s o) -> p s o", o=1))
            w2_ = scrp.tile([B, S], fp, tag="sc8")
            nc.vector.tensor_mul(w2_[:], w_[:], w_[:])
            cw = scrp.tile([B, S], fp, tag="sc8")
            nc.scalar.activation(cw[:], w2_[:], Act.Relu, bias=1.0, scale=-1.0)
            lncw = scrp.tile([B, S], fp, tag="sc8")
            nc.scalar.activation(lncw[:], cw[:], Act.Ln)
            nc.scalar.activation(pk[:, :, 1:2], lncw[:], Act.Exp, scale=0.5)

            # ================= transposed-domain samples ===================
            # packT [16, B]
            ps_pT = pp.tile([16, B], bf, tag="a")
            nc.tensor.transpose(ps_pT[:], pack[:], identB[0:B, 0:B])
            pTs = sp.tile([16, B], bf)
            nc.scalar.copy(pTs[:], ps_pT[:])
            # block-diagonal [16, SB] = pTs[q, b] * mask[q, s] (broadcast APs)
            bdiag = sp.tile([16, SB], bf)
            bd_v = bdiag.rearrange("p (s b) -> p s b", b=B)
            pT_v = pTs[:].rearrange("p (s b) -> p s b", s=1)
            mk_v = geom[0:16, G_MSK:G_MSK + S].rearrange("p (s b) -> p s b", b=1)
            pT_b, mk_b = bass.broadcast_tensor_aps(pT_v, mk_v)
            nc.vector.tensor_tensor(bd_v[:, :, :], pT_b, mk_b, op=Alu.mult)
            # uhT tiled [128, SB] (independent of pack; PE order puts it first)
            ps_uh = pp.tile([128, SB], fp, tag="big")
            nc.tensor.matmul(ps_uh[:], uhb[:], geom[0:B, G_ID8:G_ID8 + SB],
                             start=True, stop=True)
            uhsb = sp.tile([128, SB], bf)
            nc.scalar.copy(uhsb[:], ps_uh[:])
            # zT = (pick.T @ bdiag) * Vp
            ps_bd = pp.tile([128, SB], fp, tag="big")
            nc.tensor.matmul(ps_bd[:], geom[0:16, G_PICK:G_PICK + 128], bdiag[:],
                             start=True, stop=True)
            zT = sp.tile([128, SB], bf)
            nc.vector.tensor_mul(zT[:], ps_bd[:], geom[:, G_VP:G_VP + SB])
            # dp row = ones.T @ (uhT * zT)
            prodz = sp.tile([128, SB], bf)
            nc.vector.tensor_mul(prodz[:], uhsb[:], zT[:])
            ps_dp = pp.tile([1, SB], fp, tag="dp", bufs=1)
            nc.tensor.matmul(ps_dp[:], geom[:, G_ONE:G_ONE + 1], prodz[:],
                             start=True, stop=True)
            dprow = sp.tile([1, SB], bf)
            nc.scalar.copy(dprow[:], ps_dp[:])
            # dp broadcast [128, SB]
            ps_db = pp.tile([128, SB], fp, tag="big")
            nc.tensor.matmul(ps_db[:], geom[0:1, G_VP:G_VP + 128], dprow[:],
                             start=True, stop=True)
            tmp = scrp.tile([128, SB], fp, tag="big32")
            nc.vector.tensor_mul(tmp[:], uhsb[:], ps_db[:])
            smpT = sp.tile([128, SB], bf)
            nc.vector.scalar_tensor_tensor(smpT[:], tmp[:], -2.0, zT[:],
                                           op0=Alu.mult, op1=Alu.add)
            sqT = sp.tile([128, SB], bf)
            nc.scalar.activation(sqT[:], smpT[:], Act.Square)

            # ================= main matmuls + epilogue =====================
            for mc in range(4):
                pn = pp.tile([128, CLOC], fp, tag="pn")
                nc.tensor.matmul(pn[:], smpT[:, mc * 128:(mc + 1) * 128], PpTs[:],
                                 start=True, stop=True)
                pd = pp.tile([128, CLOC], fp, tag="pn")
                nc.tensor.matmul(pd[:], sqT[:, mc * 128:(mc + 1) * 128], QqTs[:],
                                 start=True, stop=True)
                lnd = scrp.tile([128, CLOC], fp, tag="ep")
                nc.scalar.activation(lnd[:], pd[:], Act.Ln)
                rd = scrp.tile([128, CLOC], fp, tag="ep")
                nc.scalar.activation(rd[:], lnd[:], Act.Exp, scale=-0.5)
                m1 = scrp.tile([128, CLOC], fp, tag="ep")
                nc.vector.tensor_mul(m1[:], pn[:], rd[:])
                o = scrp.tile([128, CLOC], fp, tag="out")
                nc.vector.tensor_add(o[:], m1[:], ps_cb[:])
                eng = nc.sync if mc % 2 == 0 else nc.gpsimd
                eng.dma_start(d_out[mc * 128:(mc + 1) * 128, :], o[:])

    with tile.TileContext(nc) as tc:
        _emit(tc)
    nc.finalize()
    return nc


def _get_nc():
    if "nc" not in _cache:
        _cache["nc"] = build_nc()
    return _cache["nc"]


def make_in_maps(inputs):
    eps_b, logu_b, geom_bf, rmask = _host_constants()
    f32 = np.float32

    # mlp pack (input-dependent bf16 hi/lo)
    mlp = np.zeros((128, M_COLS), np.float32)
    W0T = np.asarray(inputs["W0"], f32).T          # [D, H]
    W1T = np.asarray(inputs["W1"], f32).T          # [H, H]
    fT = np.asarray(inputs["features"], f32).T     # [D, B]
    W2 = np.asarray(inputs["W2"], f32)             # [1, H]
    h, l = _hilo(W0T)
    mlp[:, M_W0H:M_W0H + H] = h.astype(f32)
    mlp[:, M_W0L:M_W0L + H] = l.astype(f32)
    h, l = _hilo(W1T)
    for i in range(2):
        for j in range(2):
            blk = slice(M_W1H + (i * 2 + j) * 128, M_W1H + (i * 2 + j + 1) * 128)
            mlp[:, blk] = h[i * 128:(i + 1) * 128, j * 128:(j + 1) * 128].astype(f32)
            blk = slice(M_W1L + (i * 2 + j) * 128, M_W1L + (i * 2 + j + 1) * 128)
            mlp[:, blk] = l[i * 128:(i + 1) * 128, j * 128:(j + 1) * 128].astype(f32)
    h, l = _hilo(fT)
    mlp[:, M_FH:M_FH + B] = h.astype(f32)
    mlp[:, M_FL:M_FL + B] = l.astype(f32)
    h, l = _hilo(W2.reshape(2, 128).T)             # col j = W2[0, j*128:(j+1)*128]
    mlp[:, M_W2H:M_W2H + 2] = h.astype(f32)
    mlp[:, M_W2L:M_W2L + 2] = l.astype(f32)
    mlp_bf = _bf16(mlp)

    c32 = np.zeros((128, C_COLS), f32)
    for j in range(2):
        c32[:, C_B0 + j] = np.asarray(inputs["b0"], f32)[j * 128:(j + 1) * 128]
        c32[:, C_B1 + j] = np.asarray(inputs["b1"], f32)[j * 128:(j + 1) * 128]
    c32[0:CLOC, C_ID:C_ID + CLOC] = np.eye(CLOC)
    c32[0:1, C_ONER:C_ONER + 128] = 1.0
    c32[0:B, C_RM:C_RM + SK] = rmask

    in64 = np.zeros((B, I_COLS), f32)
    in64[:, I_EPS:I_EPS + SK] = eps_b
    in64[:, I_LOGU:I_LOGU + SK] = logu_b
    in64[:, I_FEAT:I_FEAT + D] = np.asarray(inputs["features"], f32)
    in64[:, I_B2] = float(np.asarray(inputs["b2"], f32)[0])

    com = {
        "mlp": mlp_bf,
        "geom": geom_bf,
        "c32": np.ascontiguousarray(c32),
        "in64": np.ascontiguousarray(in64),
    }
    wmu = np.asarray(inputs["W_mu"], f32)
    wkap = np.asarray(inputs["W_kappa"], f32)
    in_maps = []
    for i in range(NCORES):
        m = dict(com)
        wc = np.empty((CLOC, 2 * D), f32)
        wc[:, 0:D] = wmu[i * CLOC:(i + 1) * CLOC]
        wc[:, D:2 * D] = wkap[i * CLOC:(i + 1) * CLOC]
        m["wcls"] = wc
        in_maps.append(m)
    return in_maps


def kernel(**inputs):
    from concourse.bass_utils import run_bass_kernel_spmd

    nc = _get_nc()
    in_maps = make_in_maps(inputs)
    res = run_bass_kernel_spmd(nc, in_maps, list(range(NCORES)))
    parts = [res.results[i]["out"].reshape(S, B, CLOC) for i in range(NCORES)]
    return np.ascontiguousarray(np.concatenate(parts, axis=2).astype(np.float32))


# revision 9
# speedup vs baseline: 1.6641x; 1.6641x over previous
"""Trainium2 Bass kernel for the NonIsotropic vMF head (v2).

Contract: kernel(**inputs) takes FULL unsharded inputs (as produced by
setup_inputs()) and returns the FULL [S=8, B=64, C=1000] float32 output.

v2 strategy (vs v1 baseline):
  * 5 consolidated input DMAs issued in parallel from 5 engine queues
    (v1: ~24 serialized on Sync).
  * W0/W1/features transposed HOST-side; MLP runs as bf16 hi/lo split
    (3 bf16 matmuls emulate an fp32 matmul to ~1e-6 rel) -- kappa keeps
    fp32-grade accuracy for the rejection-accept margins.
  * Single activation table: only {exp, ln, relu, copy, identity, square}
    are used on the scalar engine; sqrt/rsqrt computed as exp(+-0.5*ln x).
  * Accept test reformulated reciprocal-free:
        margin >= 0  <=>  (E - 127*log1p(x) - logu) * denom >= 2ab.
  * Sample assembly + Householder reflection done directly in the
    transposed [D, S*B] domain with 5 small bf16 matmuls (block-diag
    broadcast trick) instead of 8 fp32 PE transposes + per-sample loops.
  * Class matmuls (num/den) in bf16 (output |val| >= 30, tol 2e-2).

RNG draws (beta/uniform/normal, key 42) are input-independent and
generated host-side exactly as the reference does, shipped as constants.
"""

import numpy as np

S, B, D, K, C, H = 8, 64, 128, 32, 1000, 256
NCORES = 8
CLOC = C // NCORES            # 125 classes per core
SB = S * B                    # 512
SK = S * K                    # 256
M1 = float(D - 1)             # 127.0
LN127 = float(np.log(M1))
LN2PI = float(np.log(2.0 * np.pi))

# ---- geom pack (bf16) [128, G_COLS]: host constants ----
G_ID = 0                      # ident        [0:128, 0:128]
G_VP = 128                    # Vp           [0:128, 128:640] row0=ones, 1..=vT
G_PICK = 640                  # pick         [0:16, 640:768]
G_ID8 = 768                   # ident64 x8   [0:64, 768:1280]
G_ONE = 1280                  # ones column  [0:128, 1280:1281]
G_MSK = 1281                  # blockdiag mask [0:16, 1281:1289]
G_COLS = 1289

# ---- mlp pack (bf16) [128, M_COLS]: input-derived hi/lo weights ----
M_W0H, M_W0L = 0, 256
M_W1H, M_W1L = 512, 1024      # four 128-col blocks each, order (i,j)
M_FH, M_FL = 1536, 1600
M_W2H, M_W2L = 1664, 1666
M_COLS = 1668

# ---- c32 pack (fp32) [128, C_COLS] ----
C_B0 = 0                      # [0:128, 0:2]
C_B1 = 2                      # [0:128, 2:4]
C_ID = 4                      # fp32 ident   [0:125, 4:129]
C_ONER = 129                  # ones row     [0:1, 129:257]
C_RM = 257                    # rmask        [0:64, 257:513]
C_COLS = 513

# ---- in64 pack (fp32) [64, I_COLS] ----
I_EPS, I_LOGU, I_FEAT, I_B2 = 0, 256, 512, 640
I_COLS = 641

_cache = {}


def _bf16(x):
    import ml_dtypes
    return np.ascontiguousarray(np.asarray(x, np.float32).astype(ml_dtypes.bfloat16))


def _hilo(x):
    import ml_dtypes
    x = np.asarray(x, np.float32)
    hi = x.astype(ml_dtypes.bfloat16)
    lo = (x - hi.astype(np.float32)).astype(ml_dtypes.bfloat16)
    return np.ascontiguousarray(hi), np.ascontiguousarray(lo)


def _host_constants():
    """RNG draws of the reference sampler (key 42) + geometry constants."""
    if "rng" in _cache:
        return _cache["rng"]
    import jax
    import jax.numpy as jnp
    import ml_dtypes

    cpu = jax.devices("cpu")[0]
    with jax.default_device(cpu):
        key = jax.random.key(42)
        k_eps, k_u, k_v = jax.random.split(key, 3)
        alpha = M1 / 2.0
        eps = np.asarray(jax.random.beta(k_eps, alpha, alpha, (K, S, B)), np.float32)
        u = jax.random.uniform(k_u, (K, S, B), jnp.float32, minval=1e-7, maxval=1.0)
        logu = np.asarray(jnp.log(u), np.float32)
        vraw = jax.random.normal(k_v, (S, B, D - 1), jnp.float32)
        vn = np.asarray(
            vraw / jnp.maximum(jnp.linalg.norm(vraw, axis=-1, keepdims=True), 1e-12),
            np.float32,
        )
    eps_b = np.ascontiguousarray(np.transpose(eps, (2, 1, 0)).reshape(B, SK))
    logu_b = np.ascontiguousarray(np.transpose(logu, (2, 1, 0)).reshape(B, SK))

    geom = np.zeros((128, G_COLS), np.float32)
    geom[:, G_ID:G_ID + 128] = np.eye(128)
    geom[0, G_VP:G_VP + SB] = 1.0
    geom[1:128, G_VP:G_VP + SB] = np.transpose(vn, (2, 0, 1)).reshape(D - 1, SB)
    # pick[2s, 0] = 1 (w row), pick[2s+1, 1:] = 1 (sm rows)
    geom[0:16:2, G_PICK] = 1.0
    geom[1:16:2, G_PICK + 1:G_PICK + 128] = 1.0
    geom[0:64, G_ID8:G_ID8 + SB] = np.concatenate([np.eye(64)] * S, axis=1)
    geom[:, G_ONE] = 1.0
    for q in range(16):
        geom[q, G_MSK + q // 2] = 1.0
    geom_bf = np.ascontiguousarray(geom.astype(ml_dtypes.bfloat16))

    rmask = np.ones((B, SK), np.float32)
    rmask[:, 0::K] = 0.0

    _cache["rng"] = (eps_b, logu_b, geom_bf, rmask)
    return _cache["rng"]


def build_nc():
    import concourse.bass as bass
    import concourse.mybir as mybir
    from concourse import bacc, tile

    fp = mybir.dt.float32
    bf = mybir.dt.bfloat16
    Alu = mybir.AluOpType
    Act = mybir.ActivationFunctionType

    nc = bacc.Bacc(None)

    d_mlp = nc.declare_dram_parameter("mlp", [128, M_COLS], bf, isOutput=False)
    d_geom = nc.declare_dram_parameter("geom", [128, G_COLS], bf, isOutput=False)
    d_c32 = nc.declare_dram_parameter("c32", [128, C_COLS], fp, isOutput=False)
    d_in64 = nc.declare_dram_parameter("in64", [B, I_COLS], fp, isOutput=False)
    d_wcls = nc.declare_dram_parameter("wcls", [CLOC, 2 * D], fp, isOutput=False)
    d_out = nc.declare_dram_parameter("out", [SB, CLOC], fp, isOutput=True)

    def _emit(tc):
        with (
            tc.tile_pool(name="w", bufs=1) as wp,
            tc.tile_pool(name="s", bufs=1) as sp,
            tc.tile_pool(name="scr", bufs=4) as scrp,
            tc.tile_pool(name="pp", bufs=2, space="PSUM") as pp,
        ):
            # ================= loads (parallel issue, 5 queues) =============
            mlp = wp.tile([128, M_COLS], bf)
            nc.sync.dma_start(mlp[:], d_mlp[:])
            wcl = wp.tile([CLOC, 2 * D], fp)
            nc.sync.dma_start(wcl[:], d_wcls[:])
            c32 = wp.tile([128, C_COLS], fp)
            nc.scalar.dma_start(c32[:], d_c32[:])
            in64 = wp.tile([B, I_COLS], fp)
            nc.scalar.dma_start(in64[:], d_in64[:])
            geom = wp.tile([128, G_COLS], bf)
            nc.gpsimd.dma_start(geom[:], d_geom[:])

            eps = in64[:, I_EPS:I_EPS + SK]
            logu = in64[:, I_LOGU:I_LOGU + SK]
            feat = in64[:, I_FEAT:I_FEAT + D]
            b2r = in64[:, I_B2:I_B2 + 1]
            wmu = wcl[:, 0:D]
            wk = wcl[:, D:2 * D]

            # ================= MLP (bf16 hi/lo) ============================
            fTh = mlp[:, M_FH:M_FH + B]
            fTl = mlp[:, M_FL:M_FL + B]
            h0r = [sp.tile([128, B], fp, name=f"h0r{j}") for j in range(2)]
            h0h = [sp.tile([128, B], bf, name=f"h0h{j}") for j in range(2)]
            h0l = [sp.tile([128, B], bf, name=f"h0l{j}") for j in range(2)]
            for j in range(2):
                w0h = mlp[:, M_W0H + j * 128:M_W0H + (j + 1) * 128]
                w0l = mlp[:, M_W0L + j * 128:M_W0L + (j + 1) * 128]
                pm = pp.tile([128, B], fp, tag="a")
                nc.tensor.matmul(pm[:], w0h, fTh, start=True, stop=False)
                nc.tensor.matmul(pm[:], w0h, fTl, start=False, stop=False)
                nc.tensor.matmul(pm[:], w0l, fTh, start=False, stop=True)
                nc.scalar.activation(h0r[j][:], pm[:], Act.Relu,
                                     bias=c32[:, C_B0 + j:C_B0 + j + 1], scale=1.0)
                nc.vector.tensor_copy(h0h[j][:], h0r[j][:])
                nc.vector.scalar_tensor_tensor(h0l[j][:], h0h[j][:], -1.0,
                                               h0r[j][:], op0=Alu.mult, op1=Alu.add)

            # ---- uh chain part 1 (needs only feat; fills scalar idle) ----
            fsq = scrp.tile([B, D], fp, tag="scBD")
            ssf = sp.tile([B, 1], fp)
            nc.scalar.activation(fsq[:], feat, Act.Square, accum_out=ssf[:])
            lnf = scrp.tile([B, 1], fp, tag="sc")
            nc.scalar.activation(lnf[:], ssf[:], Act.Ln)
            rnf = sp.tile([B, 1], fp)
            nc.scalar.activation(rnf[:], lnf[:], Act.Exp, scale=-0.5)
            em = sp.tile([B, D], fp)
            nc.vector.tensor_scalar(em[:], feat, rnf[:], -1.0, Alu.mult, Alu.mult)
            nc.vector.tensor_scalar_add(em[:, 0:1], em[:, 0:1], 1.0)

            h1r = [sp.tile([128, B], fp, name=f"h1r{j}") for j in range(2)]
            h1h = [sp.tile([128, B], bf, name=f"h1h{j}") for j in range(2)]
            h1l = [sp.tile([128, B], bf, name=f"h1l{j}") for j in range(2)]
            for j in range(2):
                pm = pp.tile([128, B], fp, tag="a")
                first = True
                for i in range(2):
                    w1h = mlp[:, M_W1H + (i * 2 + j) * 128:M_W1H + (i * 2 + j + 1) * 128]
                    w1l = mlp[:, M_W1L + (i * 2 + j) * 128:M_W1L + (i * 2 + j + 1) * 128]
                    nc.tensor.matmul(pm[:], w1h, h0h[i][:], start=first, stop=False)
                    nc.tensor.matmul(pm[:], w1h, h0l[i][:], start=False, stop=False)
                    nc.tensor.matmul(pm[:], w1l, h0h[i][:], start=False,
                                     stop=(i == 1))
                    first = False
                nc.scalar.activation(h1r[j][:], pm[:], Act.Relu,
                                     bias=c32[:, C_B1 + j:C_B1 + j + 1], scale=1.0)
                nc.vector.tensor_copy(h1h[j][:], h1r[j][:])
                nc.vector.scalar_tensor_tensor(h1l[j][:], h1h[j][:], -1.0,
                                               h1r[j][:], op0=Alu.mult, op1=Alu.add)

            # ---- uh chain part 2 ----
            esq = scrp.tile([B, D], fp, tag="scBD")
            sse = sp.tile([B, 1], fp)
            nc.scalar.activation(esq[:], em[:], Act.Square, accum_out=sse[:])
            lne = scrp.tile([B, 1], fp, tag="sc")
            nc.scalar.activation(lne[:], sse[:], Act.Ln)
            rne = sp.tile([B, 1], fp)
            nc.scalar.activation(rne[:], lne[:], Act.Exp, scale=-0.5)
            uhb = sp.tile([B, D], bf)
            nc.vector.tensor_scalar_mul(uhb[:], em[:], rne[:])

            # ---- h2 + softplus ----
            pm2 = pp.tile([B, 1], fp, tag="a")
            for j in range(2):
                w2h = mlp[:, M_W2H + j:M_W2H + j + 1]
                w2l = mlp[:, M_W2L + j:M_W2L + j + 1]
                nc.tensor.matmul(pm2[:], h1h[j][:], w2h, start=(j == 0), stop=False)
                nc.tensor.matmul(pm2[:], h1h[j][:], w2l, start=False, stop=False)
                nc.tensor.matmul(pm2[:], h1l[j][:], w2h, start=False, stop=(j == 1))
            eh2 = sp.tile([B, 1], fp)
            nc.scalar.activation(eh2[:], pm2[:], Act.Exp, bias=b2r, scale=1.0)
            kapb = sp.tile([B, 1], fp)
            nc.scalar.activation(kapb[:], eh2[:], Act.Ln, bias=1.0, scale=1.0)
            nc.vector.tensor_scalar_add(kapb[:], kapb[:], 1e-6)

            # ================= sampler scalars [B,1] =======================
            k2 = scrp.tile([B, 1], fp, tag="sc")
            nc.vector.tensor_mul(k2[:], kapb[:], kapb[:])
            nc.vector.tensor_scalar(k2[:], k2[:], 4.0, M1 * M1, Alu.mult, Alu.add)
            lnk2 = scrp.tile([B, 1], fp, tag="sc")
            nc.scalar.activation(lnk2[:], k2[:], Act.Ln)
            sq = sp.tile([B, 1], fp)
            nc.scalar.activation(sq[:], lnk2[:], Act.Exp, scale=0.5)
            b_ = sp.tile([B, 1], fp)
            nc.vector.scalar_tensor_tensor(b_[:], kapb[:], -2.0, sq[:],
                                           op0=Alu.mult, op1=Alu.add)
            nc.vector.tensor_scalar_mul(b_[:], b_[:], 1.0 / M1)
            a_ = sp.tile([B, 1], fp)
            nc.vector.scalar_tensor_tensor(a_[:], kapb[:], 2.0, sq[:],
                                           op0=Alu.mult, op1=Alu.add)
            nc.vector.tensor_scalar(a_[:], a_[:], M1, 0.25, Alu.add, Alu.mult)
            ab = sp.tile([B, 1], fp)
            nc.vector.tensor_mul(ab[:], a_[:], b_[:])
            opb = scrp.tile([B, 1], fp, tag="sc")
            nc.vector.tensor_scalar_add(opb[:], b_[:], 1.0)
            r1pb = scrp.tile([B, 1], fp, tag="sc")
            nc.vector.reciprocal(r1pb[:], opb[:])
            d_ = sp.tile([B, 1], fp)
            nc.vector.scalar_tensor_tensor(d_[:], ab[:], 4.0, r1pb[:],
                                           op0=Alu.mult, op1=Alu.mult)
            nc.vector.tensor_scalar_add(d_[:], d_[:], -M1 * LN127)
            l2ab = scrp.tile([B, 1], fp, tag="sc")
            nc.scalar.activation(l2ab[:], ab[:], Act.Ln, scale=2.0)
            E = sp.tile([B, 1], fp)
            nc.vector.scalar_tensor_tensor(E[:], l2ab[:], M1, d_[:],
                                           op0=Alu.mult, op1=Alu.add)
            p2ab = sp.tile([B, 1], fp)
            nc.vector.tensor_scalar_mul(p2ab[:], ab[:], 2.0)
            ncm = sp.tile([B, 1], fp)
            nc.vector.tensor_scalar_add(ncm[:], b_[:], -1.0)
            ncp = sp.tile([B, 1], fp)
            nc.vector.tensor_scalar(ncp[:], b_[:], -1.0, -1.0, Alu.mult, Alu.add)

            # ================= class shard stats (gpsimd + scalar) ========
            kapc = sp.tile([CLOC, D], fp)
            nc.vector.tensor_scalar_max(kapc[:], wk, 0.1)
            msq = scrp.tile([CLOC, D], fp, tag="scCD")
            ssm = sp.tile([CLOC, 1], fp)
            nc.scalar.activation(msq[:], wmu, Act.Square, accum_out=ssm[:])
            lnsm = scrp.tile([CLOC, 1], fp, tag="scC")
            nc.scalar.activation(lnsm[:], ssm[:], Act.Ln)
            rnm = sp.tile([CLOC, 1], fp)
            nc.scalar.activation(rnm[:], lnsm[:], Act.Exp, scale=-0.5)
            scm = sp.tile([CLOC, D], fp)
            nc.vector.tensor_mul(scm[:], wmu, kapc[:])
            nc.vector.tensor_scalar_mul(scm[:], scm[:], rnm[:])
            csq = scrp.tile([CLOC, D], fp, tag="scCD")
            ssc = sp.tile([CLOC, 1], fp)
            nc.scalar.activation(csq[:], scm[:], Act.Square, accum_out=ssc[:])
            Ppb = sp.tile([CLOC, D], bf)
            nc.vector.tensor_mul(Ppb[:], kapc[:], scm[:])
            Qqb = sp.tile([CLOC, D], bf)
            nc.vector.tensor_mul(Qqb[:], kapc[:], kapc[:])
            lkt = scrp.tile([CLOC, D], fp, tag="scCD")
            slk = sp.tile([CLOC, 1], fp)
            nc.scalar.activation(lkt[:], kapc[:], Act.Ln, accum_out=slk[:])
            G = sp.tile([CLOC, 1], fp)
            nc.gpsimd.tensor_scalar_add(G[:], ssc[:], 63.0 * 63.0)
            lnG = sp.tile([CLOC, 1], fp)
            nc.scalar.activation(lnG[:], G[:], Act.Ln)
            eta = sp.tile([CLOC, 1], fp)
            nc.scalar.activation(eta[:], lnG[:], Act.Exp, scale=0.5)
            etap = scrp.tile([CLOC, 1], fp, tag="scC")
            nc.gpsimd.tensor_scalar_add(etap[:], eta[:], 63.0)
            l63 = scrp.tile([CLOC, 1], fp, tag="scC")
            nc.scalar.activation(l63[:], etap[:], Act.Ln)
            lnssc = scrp.tile([CLOC, 1], fp, tag="scC")
            nc.scalar.activation(lnssc[:], ssc[:], Act.Ln)
            c1 = scrp.tile([CLOC, 1], fp, tag="scC")
            nc.gpsimd.tensor_scalar_mul(c1[:], l63[:], 63.0)
            nc.gpsimd.tensor_sub(c1[:], c1[:], eta[:])
            c2 = scrp.tile([CLOC, 1], fp, tag="scC")
            nc.gpsimd.tensor_scalar_mul(c2[:], lnssc[:], -0.5)
            nc.gpsimd.tensor_add(c2[:], c2[:], slk[:])
            nc.gpsimd.tensor_add(c1[:], c1[:], c2[:])
            cst = sp.tile([CLOC, 1], fp)
            nc.gpsimd.tensor_scalar_mul(cst[:], lnG[:], 0.25)
            nc.gpsimd.tensor_add(cst[:], cst[:], c1[:])
            nc.gpsimd.tensor_scalar_add(cst[:], cst[:], -63.5 * LN2PI)

            # class transposes + const broadcast
            identB = geom[:, G_ID:G_ID + 128]
            ps = pp.tile([128, CLOC], bf, tag="a")
            nc.tensor.transpose(ps[:], Ppb[:], identB[0:CLOC, 0:CLOC])
            PpTs = sp.tile([128, CLOC], bf)
            nc.scalar.copy(PpTs[:], ps[:])
            ps = pp.tile([128, CLOC], bf, tag="a")
            nc.tensor.transpose(ps[:], Qqb[:], identB[0:CLOC, 0:CLOC])
            QqTs = sp.tile([128, CLOC], bf)
            nc.vector.tensor_copy(QqTs[:], ps[:])
            ps = pp.tile([1, CLOC], fp, tag="a")
            nc.tensor.transpose(ps[:], cst[:], c32[0:CLOC, C_ID:C_ID + CLOC])
            cstTs = sp.tile([1, CLOC], fp)
            nc.scalar.copy(cstTs[:], ps[:])
            ps_cb = pp.tile([128, CLOC], fp, tag="cb", bufs=1)
            nc.tensor.matmul(ps_cb[:], c32[0:1, C_ONER:C_ONER + 128], cstTs[:],
                             start=True, stop=True)

            # ================= accept [B, SK], split V/G ===================
            x_ = sp.tile([B, SK], fp)
            den = sp.tile([B, SK], fp)
            u_ = sp.tile([B, SK], fp)
            acc = sp.tile([B, SK], fp)
            A = sp.tile([B, SK], fp)
            HF = SK // 2
            nc.vector.tensor_scalar_mul(x_[:], eps, ncm[:])
            nc.vector.tensor_scalar_add(den[:], x_[:], 1.0)
            nc.vector.tensor_scalar(u_[:], x_[:], 1.0 / 3.0, -0.5, Alu.mult, Alu.add)
            nc.vector.tensor_mul(u_[:], u_[:], x_[:])
            nc.vector.scalar_tensor_tensor(acc[:], u_[:], 1.0, x_[:],
                                           op0=Alu.add, op1=Alu.mult)
            nc.vector.tensor_scalar(acc[:], acc[:], -M1, E[:], Alu.mult, Alu.add)
            nc.vector.tensor_sub(acc[:], acc[:], logu)
            nc.vector.tensor_mul(acc[:], acc[:], den[:])
            nc.vector.tensor_scalar(A[:], acc[:], p2ab[:], None, Alu.is_ge)

            P = sp.tile([B, SK], fp)
            nc.vector.tensor_tensor_scan(P[:], c32[0:B, C_RM:C_RM + SK], A[:],
                                         0.0, op0=Alu.mult, op1=Alu.max)
            Pm1 = sp.tile([B, SK], fp)
            nc.vector.tensor_copy(Pm1[:, 1:SK], P[:, 0:SK - 1])
            Pm1_v = Pm1.rearrange("p (s r) -> p s r", r=K)
            nc.gpsimd.memset(Pm1_v[:, :, 0:1], 0.0)
            first = sp.tile([B, SK], fp)
            nc.vector.tensor_sub(first[:], P[:], Pm1[:])
            prod = sp.tile([B, SK], fp)
            nc.vector.tensor_mul(prod[:], eps, first[:])
            esel = sp.tile([B, S], fp)
            nc.vector.tensor_reduce(esel[:],
                                    prod.rearrange("p (s r) -> p s r", r=K),
                                    axis=mybir.AxisListType.X, op=Alu.add)
            fb = scrp.tile([B, S], fp, tag="sc8")
            nc.vector.scalar_tensor_tensor(fb[:], P[:, K - 1::K], 1.0, eps[:, 0::K],
                                           op0=Alu.subtract, op1=Alu.mult)
            nc.vector.tensor_sub(esel[:], esel[:], fb[:])

            # ================= w, sm -> interleaved pack [B, 16] ===========
            pack = sp.tile([B, 2 * S], bf)
            pk = pack.rearrange("p (s two) -> p s two", two=2)
            n1 = scrp.tile([B, S], fp, tag="sc8")
            nc.vector.tensor_scalar(n1[:], esel[:], ncp[:], 1.0, Alu.mult, Alu.add)
            d1 = scrp.tile([B, S], fp, tag="sc8")
            nc.vector.tensor_scalar(d1[:], esel[:], ncm[:], 1.0, Alu.mult, Alu.add)
            rd1 = scrp.tile([B, S], fp, tag="sc8")
            nc.vector.reciprocal(rd1[:], d1[:])
            w_ = sp.tile([B, S], fp)
            nc.vector.tensor_mul(w_[:], n1[:], rd1[:])
            nc.vector.tensor_copy(pk[:, :, 0:1], w_.rearrange("p (s o) -> p s o", o=1))
            w2_ = scrp.tile([B, S], fp, tag="sc8")
            nc.vector.tensor_mul(w2_[:], w_[:], w_[:])
            cw = scrp.tile([B, S], fp, tag="sc8")
            nc.scalar.activation(cw[:], w2_[:], Act.Relu, bias=1.0, scale=-1.0)
            lncw = scrp.tile([B, S], fp, tag="sc8")
            nc.scalar.activation(lncw[:], cw[:], Act.Ln)
            nc.scalar.activation(pk[:, :, 1:2], lncw[:], Act.Exp, scale=0.5)

            # ================= transposed-domain samples ===================
            # packT [16, B]
            ps_pT = pp.tile([16, B], bf, tag="a")
            nc.tensor.transpose(ps_pT[:], pack[:], identB[0:B, 0:B])
            pTs = sp.tile([16, B], bf)
            nc.scalar.copy(pTs[:], ps_pT[:])
            # block-diagonal [16, SB] = pTs[q, b] * mask[q, s] (broadcast APs)
            bdiag = sp.tile([16, SB], bf)
            bd_v = bdiag.rearrange("p (s b) -> p s b", b=B)
            pT_v = pTs[:].rearrange("p (s b) -> p s b", s=1)
            mk_v = geom[0:16, G_MSK:G_MSK + S].rearrange("p (s b) -> p s b", b=1)
            pT_b, mk_b = bass.broadcast_tensor_aps(pT_v, mk_v)
            nc.vector.tensor_tensor(bd_v[:, :, :], pT_b, mk_b, op=Alu.mult)
            # uhT tiled [128, SB] (independent of pack; PE order puts it first)
            ps_uh = pp.tile([128, SB], fp, tag="big")
            nc.tensor.matmul(ps_uh[:], uhb[:], geom[0:B, G_ID8:G_ID8 + SB],
                             start=True, stop=True)
            uhsb = sp.tile([128, SB], bf)
            nc.scalar.copy(uhsb[:], ps_uh[:])
            # zT = (pick.T @ bdiag) * Vp
            ps_bd = pp.tile([128, SB], fp, tag="big")
            nc.tensor.matmul(ps_bd[:], geom[0:16, G_PICK:G_PICK + 128], bdiag[:],
                             start=True, stop=True)
            zT = sp.tile([128, SB], bf)
            nc.vector.tensor_mul(zT[:], ps_bd[:], geom[:, G_VP:G_VP + SB])
            # dp row = ones.T @ (uhT * zT)
            prodz = sp.tile([128, SB], bf)
            nc.vector.tensor_mul(prodz[:], uhsb[:], zT[:])
            ps_dp = pp.tile([1, SB], fp, tag="dp", bufs=1)
            nc.tensor.matmul(ps_dp[:], geom[:, G_ONE:G_ONE + 1], prodz[:],
                             start=True, stop=True)
            dprow = sp.tile([1, SB], bf)
            nc.scalar.copy(dprow[:], ps_dp[:])
            # dp broadcast [128, SB]
            ps_db = pp.tile([128, SB], fp, tag="big")
            nc.tensor.matmul(ps_db[:], geom[0:1, G_VP:G_VP + 128], dprow[:],
                             start=True, stop=True)
            tmp = scrp.tile([128, SB], fp, tag="big32")
            nc.vector.tensor_mul(tmp[:], uhsb[:], ps_db[:])
            smpT = sp.tile([128, SB], bf)
            nc.vector.scalar_tensor_tensor(smpT[:], tmp[:], -2.0, zT[:],
                                           op0=Alu.mult, op1=Alu.add)
            sqT = sp.tile([128, SB], bf)
            nc.vector.tensor_mul(sqT[:], smpT[:], smpT[:])

            # ================= main matmuls + epilogue =====================
            for mc in range(4):
                pn = pp.tile([128, CLOC], fp, tag="pn")
                nc.tensor.matmul(pn[:], smpT[:, mc * 128:(mc + 1) * 128], PpTs[:],
                                 start=True, stop=True)
                pd = pp.tile([128, CLOC], fp, tag="pn")
                nc.tensor.matmul(pd[:], sqT[:, mc * 128:(mc + 1) * 128], QqTs[:],
                                 start=True, stop=True)
                lnd = scrp.tile([128, CLOC], fp, tag="ep")
                nc.scalar.activation(lnd[:], pd[:], Act.Ln)
                rd = scrp.tile([128, CLOC], fp, tag="ep")
                nc.scalar.activation(rd[:], lnd[:], Act.Exp, scale=-0.5)
                m1 = scrp.tile([128, CLOC], fp, tag="ep")
                nc.vector.tensor_mul(m1[:], pn[:], rd[:])
                o = scrp.tile([128, CLOC], fp, tag="out")
                nc.vector.tensor_add(o[:], m1[:], ps_cb[:])
                eng = nc.sync if mc % 2 == 0 else nc.gpsimd
                eng.dma_start(d_out[mc * 128:(mc + 1) * 128, :], o[:])

    with tile.TileContext(nc) as tc:
        _emit(tc)
    nc.finalize()
    # All scalar-engine activations use funcs in natural_log_exp_and_others
    # (exp, ln, relu, copy, identity, square).  The auto-inserter picks
    # first-match tables and thrashes between exp_and_others and natural_log
    # (1283ns per load); rewrite to the shared table and drop redundant loads.
    from concourse.hw_specs import get_activation_tables
    tabs = list(get_activation_tables(nc.m.arch).items())
    lnexp = next(i for i, (n, fs) in enumerate(tabs)
                 if n == "natural_log_exp_and_others")
    seen = False
    for blk in nc.m.functions[0].blocks:
        keep = []
        for ins in blk.instructions:
            if isinstance(ins, mybir.InstLoadActFuncSet):
                if seen:
                    continue
                ins.act_func_set_id = lnexp
                seen = True
            keep.append(ins)
        blk.instructions[:] = keep
    return nc


def _get_nc():
    if "nc" not in _cache:
        _cache["nc"] = build_nc()
    return _cache["nc"]


def make_in_maps(inputs):
    eps_b, logu_b, geom_bf, rmask = _host_constants()
    f32 = np.float32

    # mlp pack (input-dependent bf16 hi/lo)
    mlp = np.zeros((128, M_COLS), np.float32)
    W0T = np.asarray(inputs["W0"], f32).T          # [D, H]
    W1T = np.asarray(inputs["W1"], f32).T          # [H, H]
    fT = np.asarray(inputs["features"], f32).T     # [D, B]
    W2 = np.asarray(inputs["W2"], f32)             # [1, H]
    h, l = _hilo(W0T)
    mlp[:, M_W0H:M_W0H + H] = h.astype(f32)
    mlp[:, M_W0L:M_W0L + H] = l.astype(f32)
    h, l = _hilo(W1T)
    for i in range(2):
        for j in range(2):
            blk = slice(M_W1H + (i * 2 + j) * 128, M_W1H + (i * 2 + j + 1) * 128)
            mlp[:, blk] = h[i * 128:(i + 1) * 128, j * 128:(j + 1) * 128].astype(f32)
            blk = slice(M_W1L + (i * 2 + j) * 128, M_W1L + (i * 2 + j + 1) * 128)
            mlp[:, blk] = l[i * 128:(i + 1) * 128, j * 128:(j + 1) * 128].astype(f32)
    h, l = _hilo(fT)
    mlp[:, M_FH:M_FH + B] = h.astype(f32)
    mlp[:, M_FL:M_FL + B] = l.astype(f32)
    h, l = _hilo(W2.reshape(2, 128).T)             # col j = W2[0, j*128:(j+1)*128]
    mlp[:, M_W2H:M_W2H + 2] = h.astype(f32)
    mlp[:, M_W2L:M_W2L + 2] = l.astype(f32)
    mlp_bf = _bf16(mlp)

    c32 = np.zeros((128, C_COLS), f32)
    for j in range(2):
        c32[:, C_B0 + j] = np.asarray(inputs["b0"], f32)[j * 128:(j + 1) * 128]
        c32[:, C_B1 + j] = np.asarray(inputs["b1"], f32)[j * 128:(j + 1) * 128]
    c32[0:CLOC, C_ID:C_ID + CLOC] = np.eye(CLOC)
    c32[0:1, C_ONER:C_ONER + 128] = 1.0
    c32[0:B, C_RM:C_RM + SK] = rmask

    in64 = np.zeros((B, I_COLS), f32)
    in64[:, I_EPS:I_EPS + SK] = eps_b
    in64[:, I_LOGU:I_LOGU + SK] = logu_b
    in64[:, I_FEAT:I_FEAT + D] = np.asarray(inputs["features"], f32)
    in64[:, I_B2] = float(np.asarray(inputs["b2"], f32)[0])

    com = {
        "mlp": mlp_bf,
        "geom": geom_bf,
        "c32": np.ascontiguousarray(c32),
        "in64": np.ascontiguousarray(in64),
    }
    wmu = np.asarray(inputs["W_mu"], f32)
    wkap = np.asarray(inputs["W_kappa"], f32)
    in_maps = []
    for i in range(NCORES):
        m = dict(com)
        wc = np.empty((CLOC, 2 * D), f32)
        wc[:, 0:D] = wmu[i * CLOC:(i + 1) * CLOC]
        wc[:, D:2 * D] = wkap[i * CLOC:(i + 1) * CLOC]
        m["wcls"] = wc
        in_maps.append(m)
    return in_maps


def kernel(**inputs):
    from concourse.bass_utils import run_bass_kernel_spmd

    nc = _get_nc()
    in_maps = make_in_maps(inputs)
    res = run_bass_kernel_spmd(nc, in_maps, list(range(NCORES)))
    parts = [res.results[i]["out"].reshape(S, B, CLOC) for i in range(NCORES)]
    return np.ascontiguousarray(np.concatenate(parts, axis=2).astype(np.float32))


# revision 13
# speedup vs baseline: 1.6995x; 1.0213x over previous
"""Trainium2 Bass kernel for the NonIsotropic vMF head (v2).

Contract: kernel(**inputs) takes FULL unsharded inputs (as produced by
setup_inputs()) and returns the FULL [S=8, B=64, C=1000] float32 output.

v2 strategy (vs v1 baseline):
  * 5 consolidated input DMAs issued in parallel from 5 engine queues
    (v1: ~24 serialized on Sync).
  * W0/W1/features transposed HOST-side; MLP runs as bf16 hi/lo split
    (3 bf16 matmuls emulate an fp32 matmul to ~1e-6 rel) -- kappa keeps
    fp32-grade accuracy for the rejection-accept margins.
  * Single activation table: only {exp, ln, relu, copy, identity, square}
    are used on the scalar engine; sqrt/rsqrt computed as exp(+-0.5*ln x).
  * Accept test reformulated reciprocal-free:
        margin >= 0  <=>  (E - 127*log1p(x) - logu) * denom >= 2ab.
  * Sample assembly + Householder reflection done directly in the
    transposed [D, S*B] domain with 5 small bf16 matmuls (block-diag
    broadcast trick) instead of 8 fp32 PE transposes + per-sample loops.
  * Class matmuls (num/den) in bf16 (output |val| >= 30, tol 2e-2).

RNG draws (beta/uniform/normal, key 42) are input-independent and
generated host-side exactly as the reference does, shipped as constants.
"""

import numpy as np

S, B, D, K, C, H = 8, 64, 128, 32, 1000, 256
NCORES = 8
CLOC = C // NCORES            # 125 classes per core
SB = S * B                    # 512
SK = S * K                    # 256
M1 = float(D - 1)             # 127.0
LN127 = float(np.log(M1))
LN2PI = float(np.log(2.0 * np.pi))

# ---- geom pack (bf16) [128, G_COLS]: host constants ----
G_ID = 0                      # ident        [0:128, 0:128]
G_VP = 128                    # Vp           [0:128, 128:640] row0=ones, 1..=vT
G_PICK = 640                  # pick         [0:16, 640:768]
G_ID8 = 768                   # ident64 x8   [0:64, 768:1280]
G_ONE = 1280                  # ones column  [0:128, 1280:1281]
G_MSK = 1281                  # blockdiag mask [0:16, 1281:1289]
G_ONES = 1289                 # all-ones [0:128, 1289:1417] (dp broadcast)
G_COLS = 1417

# ---- mlp packs (bf16): input-derived hi/lo weights ----
# mlp0 [128, 644]: W0 + features + W2 (everything h0/h2 needs)
M_W0H, M_W0L = 0, 256
M_FH, M_FL = 512, 576
M_W2H, M_W2L = 640, 642
M0_COLS = 644
# mlp1 [128, 1024]: four 128-col W1 blocks, order (i,j), hi then lo
M_W1H, M_W1L = 0, 512
M1_COLS = 1024

# ---- c32 pack (fp32) [128, C_COLS] ----
C_B0 = 0                      # [0:128, 0:2]
C_B1 = 2                      # [0:128, 2:4]
C_ID = 4                      # fp32 ident   [0:125, 4:129]
C_ONER = 129                  # ones row     [0:1, 129:257]
C_RM = 257                    # rmask        [0:64, 257:513]
C_COLS = 513

# ---- in64 pack (fp32) [64, I_COLS] ----
I_EPS, I_LOGU, I_FEAT, I_B2 = 0, 256, 512, 640
I_COLS = 641

_cache = {}


def _bf16(x):
    import ml_dtypes
    return np.ascontiguousarray(np.asarray(x, np.float32).astype(ml_dtypes.bfloat16))


def _hilo(x):
    import ml_dtypes
    x = np.asarray(x, np.float32)
    hi = x.astype(ml_dtypes.bfloat16)
    lo = (x - hi.astype(np.float32)).astype(ml_dtypes.bfloat16)
    return np.ascontiguousarray(hi), np.ascontiguousarray(lo)


def _host_constants():
    """RNG draws of the reference sampler (key 42) + geometry constants."""
    if "rng" in _cache:
        return _cache["rng"]
    import jax
    import jax.numpy as jnp
    import ml_dtypes

    cpu = jax.devices("cpu")[0]
    with jax.default_device(cpu):
        key = jax.random.key(42)
        k_eps, k_u, k_v = jax.random.split(key, 3)
        alpha = M1 / 2.0
        eps = np.asarray(jax.random.beta(k_eps, alpha, alpha, (K, S, B)), np.float32)
        u = jax.random.uniform(k_u, (K, S, B), jnp.float32, minval=1e-7, maxval=1.0)
        logu = np.asarray(jnp.log(u), np.float32)
        vraw = jax.random.normal(k_v, (S, B, D - 1), jnp.float32)
        vn = np.asarray(
            vraw / jnp.maximum(jnp.linalg.norm(vraw, axis=-1, keepdims=True), 1e-12),
            np.float32,
        )
    eps_b = np.ascontiguousarray(np.transpose(eps, (2, 1, 0)).reshape(B, SK))
    logu_b = np.ascontiguousarray(np.transpose(logu, (2, 1, 0)).reshape(B, SK))

    geom = np.zeros((128, G_COLS), np.float32)
    geom[:, G_ID:G_ID + 128] = np.eye(128)
    geom[0, G_VP:G_VP + SB] = 1.0
    geom[1:128, G_VP:G_VP + SB] = np.transpose(vn, (2, 0, 1)).reshape(D - 1, SB)
    # pick[2s, 0] = 1 (w row), pick[2s+1, 1:] = 1 (sm rows)
    geom[0:16:2, G_PICK] = 1.0
    geom[1:16:2, G_PICK + 1:G_PICK + 128] = 1.0
    geom[0:64, G_ID8:G_ID8 + SB] = np.concatenate([np.eye(64)] * S, axis=1)
    geom[:, G_ONE] = 1.0
    for q in range(16):
        geom[q, G_MSK + q // 2] = 1.0
    geom[:, G_ONES:G_ONES + 128] = 1.0
    geom_bf = np.ascontiguousarray(geom.astype(ml_dtypes.bfloat16))

    rmask = np.ones((B, SK), np.float32)
    rmask[:, 0::K] = 0.0

    _cache["rng"] = (eps_b, logu_b, geom_bf, rmask)
    return _cache["rng"]


def build_nc():
    import concourse.bass as bass
    import concourse.mybir as mybir
    from concourse import bacc, tile

    fp = mybir.dt.float32
    bf = mybir.dt.bfloat16
    Alu = mybir.AluOpType
    Act = mybir.ActivationFunctionType

    nc = bacc.Bacc(None)

    d_mlp0 = nc.declare_dram_parameter("mlp0", [128, M0_COLS], bf, isOutput=False)
    d_mlp1 = nc.declare_dram_parameter("mlp1", [128, M1_COLS], bf, isOutput=False)
    d_geom = nc.declare_dram_parameter("geom", [128, G_COLS], bf, isOutput=False)
    d_c32 = nc.declare_dram_parameter("c32", [128, C_COLS], fp, isOutput=False)
    d_in64 = nc.declare_dram_parameter("in64", [B, I_COLS], fp, isOutput=False)
    d_wcls = nc.declare_dram_parameter("wcls", [CLOC, 2 * D], fp, isOutput=False)
    d_out = nc.declare_dram_parameter("out", [SB, CLOC], fp, isOutput=True)

    def _emit(tc):
        with (
            tc.tile_pool(name="w", bufs=1) as wp,
            tc.tile_pool(name="s", bufs=1) as sp,
            tc.tile_pool(name="scr", bufs=4) as scrp,
            tc.tile_pool(name="pp", bufs=2, space="PSUM") as pp,
        ):
            # ================= loads (parallel issue, 5 queues) =============
            mlp0 = wp.tile([128, M0_COLS], bf)
            nc.sync.dma_start(mlp0[:], d_mlp0[:])
            mlp1 = wp.tile([128, M1_COLS], bf)
            nc.sync.dma_start(mlp1[:], d_mlp1[:])
            wcl = wp.tile([CLOC, 2 * D], fp)
            nc.sync.dma_start(wcl[:], d_wcls[:])
            c32 = wp.tile([128, C_COLS], fp)
            nc.scalar.dma_start(c32[:], d_c32[:])
            in64 = wp.tile([B, I_COLS], fp)
            nc.scalar.dma_start(in64[:], d_in64[:])
            geom = wp.tile([128, G_COLS], bf)
            nc.gpsimd.dma_start(geom[:], d_geom[:])

            eps = in64[:, I_EPS:I_EPS + SK]
            logu = in64[:, I_LOGU:I_LOGU + SK]
            feat = in64[:, I_FEAT:I_FEAT + D]
            b2r = in64[:, I_B2:I_B2 + 1]
            wmu = wcl[:, 0:D]
            wk = wcl[:, D:2 * D]

            # ================= MLP (bf16 hi/lo) ============================
            fTh = mlp0[:, M_FH:M_FH + B]
            fTl = mlp0[:, M_FL:M_FL + B]
            h0r = [sp.tile([128, B], fp, name=f"h0r{j}") for j in range(2)]
            h0h = [sp.tile([128, B], bf, name=f"h0h{j}") for j in range(2)]
            h0l = [sp.tile([128, B], bf, name=f"h0l{j}") for j in range(2)]
            for j in range(2):
                w0h = mlp0[:, M_W0H + j * 128:M_W0H + (j + 1) * 128]
                w0l = mlp0[:, M_W0L + j * 128:M_W0L + (j + 1) * 128]
                pm = pp.tile([128, B], fp, tag="a")
                nc.tensor.matmul(pm[:], w0h, fTh, start=True, stop=False)
                nc.tensor.matmul(pm[:], w0h, fTl, start=False, stop=False)
                nc.tensor.matmul(pm[:], w0l, fTh, start=False, stop=True)
                nc.scalar.activation(h0h[j][:], pm[:], Act.Relu,
                                     bias=c32[:, C_B0 + j:C_B0 + j + 1], scale=1.0)
                nc.scalar.activation(h0r[j][:], pm[:], Act.Relu,
                                     bias=c32[:, C_B0 + j:C_B0 + j + 1], scale=1.0)
                nc.vector.scalar_tensor_tensor(h0l[j][:], h0h[j][:], -1.0,
                                               h0r[j][:], op0=Alu.mult, op1=Alu.add)

            # ---- uh chain part 1 (needs only feat; fills scalar idle) ----
            fsq = scrp.tile([B, D], fp, tag="scBD")
            ssf = sp.tile([B, 1], fp)
            nc.scalar.activation(fsq[:], feat, Act.Square, accum_out=ssf[:])
            lnf = scrp.tile([B, 1], fp, tag="sc")
            nc.scalar.activation(lnf[:], ssf[:], Act.Ln)
            rnf = sp.tile([B, 1], fp)
            nc.scalar.activation(rnf[:], lnf[:], Act.Exp, scale=-0.5)
            em = sp.tile([B, D], fp)
            nc.vector.tensor_scalar(em[:], feat, rnf[:], -1.0, Alu.mult, Alu.mult)
            nc.vector.tensor_scalar_add(em[:, 0:1], em[:, 0:1], 1.0)

            h1r = [sp.tile([128, B], fp, name=f"h1r{j}") for j in range(2)]
            h1h = [sp.tile([128, B], bf, name=f"h1h{j}") for j in range(2)]
            h1l = [sp.tile([128, B], bf, name=f"h1l{j}") for j in range(2)]
            w1b = lambda hl, i, j: mlp1[:, hl + (i * 2 + j) * 128:hl + (i * 2 + j + 1) * 128]
            pmj = [pp.tile([128, B], fp, tag="a", name=f"pmh1{j}") for j in range(2)]
            for j in range(2):
                nc.tensor.matmul(pmj[j][:], w1b(M_W1H, 0, j), h0h[0][:], start=True, stop=False)
                nc.tensor.matmul(pmj[j][:], w1b(M_W1L, 0, j), h0h[0][:], start=False, stop=False)
                nc.tensor.matmul(pmj[j][:], w1b(M_W1H, 1, j), h0h[1][:], start=False, stop=False)
                nc.tensor.matmul(pmj[j][:], w1b(M_W1L, 1, j), h0h[1][:], start=False, stop=False)
                nc.tensor.matmul(pmj[j][:], w1b(M_W1H, 0, j), h0l[0][:], start=False, stop=False)
                nc.tensor.matmul(pmj[j][:], w1b(M_W1H, 1, j), h0l[1][:], start=False, stop=True)
            for j in range(2):
                nc.scalar.activation(h1h[j][:], pmj[j][:], Act.Relu,
                                     bias=c32[:, C_B1 + j:C_B1 + j + 1], scale=1.0)
                nc.scalar.activation(h1r[j][:], pmj[j][:], Act.Relu,
                                     bias=c32[:, C_B1 + j:C_B1 + j + 1], scale=1.0)
                nc.vector.scalar_tensor_tensor(h1l[j][:], h1h[j][:], -1.0,
                                               h1r[j][:], op0=Alu.mult, op1=Alu.add)

            # ---- uh chain part 2 ----
            esq = scrp.tile([B, D], fp, tag="scBD")
            sse = sp.tile([B, 1], fp)
            nc.scalar.activation(esq[:], em[:], Act.Square, accum_out=sse[:])
            lne = scrp.tile([B, 1], fp, tag="sc")
            nc.scalar.activation(lne[:], sse[:], Act.Ln)
            rne = sp.tile([B, 1], fp)
            nc.scalar.activation(rne[:], lne[:], Act.Exp, scale=-0.5)
            uhb = sp.tile([B, D], bf)
            nc.vector.tensor_scalar_mul(uhb[:], em[:], rne[:])

            # ---- h2 + softplus ----
            pm2 = pp.tile([B, 1], fp, tag="a")
            for j in range(2):
                w2h = mlp0[:, M_W2H + j:M_W2H + j + 1]
                w2l = mlp0[:, M_W2L + j:M_W2L + j + 1]
                nc.tensor.matmul(pm2[:], h1h[j][:], w2h, start=(j == 0), stop=False)
                nc.tensor.matmul(pm2[:], h1h[j][:], w2l, start=False, stop=False)
                nc.tensor.matmul(pm2[:], h1l[j][:], w2h, start=False, stop=(j == 1))
            eh2 = sp.tile([B, 1], fp)
            nc.scalar.activation(eh2[:], pm2[:], Act.Exp, bias=b2r, scale=1.0)
            kapb = sp.tile([B, 1], fp)
            nc.scalar.activation(kapb[:], eh2[:], Act.Ln, bias=1.0, scale=1.0)
            nc.vector.tensor_scalar_add(kapb[:], kapb[:], 1e-6)

            # ================= sampler scalars [B,1] =======================
            k2 = scrp.tile([B, 1], fp, tag="sc")
            nc.vector.tensor_mul(k2[:], kapb[:], kapb[:])
            nc.vector.tensor_scalar(k2[:], k2[:], 4.0, M1 * M1, Alu.mult, Alu.add)
            lnk2 = scrp.tile([B, 1], fp, tag="sc")
            nc.scalar.activation(lnk2[:], k2[:], Act.Ln)
            sq = sp.tile([B, 1], fp)
            nc.scalar.activation(sq[:], lnk2[:], Act.Exp, scale=0.5)
            b_ = sp.tile([B, 1], fp)
            nc.vector.scalar_tensor_tensor(b_[:], kapb[:], -2.0, sq[:],
                                           op0=Alu.mult, op1=Alu.add)
            nc.vector.tensor_scalar_mul(b_[:], b_[:], 1.0 / M1)
            a_ = sp.tile([B, 1], fp)
            nc.vector.scalar_tensor_tensor(a_[:], kapb[:], 2.0, sq[:],
                                           op0=Alu.mult, op1=Alu.add)
            nc.vector.tensor_scalar(a_[:], a_[:], M1, 0.25, Alu.add, Alu.mult)
            ab = sp.tile([B, 1], fp)
            nc.vector.tensor_mul(ab[:], a_[:], b_[:])
            opb = scrp.tile([B, 1], fp, tag="sc")
            nc.vector.tensor_scalar_add(opb[:], b_[:], 1.0)
            r1pb = scrp.tile([B, 1], fp, tag="sc")
            nc.vector.reciprocal(r1pb[:], opb[:])
            d_ = sp.tile([B, 1], fp)
            nc.vector.scalar_tensor_tensor(d_[:], ab[:], 4.0, r1pb[:],
                                           op0=Alu.mult, op1=Alu.mult)
            nc.vector.tensor_scalar_add(d_[:], d_[:], -M1 * LN127)
            l2ab = scrp.tile([B, 1], fp, tag="sc")
            nc.scalar.activation(l2ab[:], ab[:], Act.Ln, scale=2.0)
            E = sp.tile([B, 1], fp)
            nc.vector.scalar_tensor_tensor(E[:], l2ab[:], M1, d_[:],
                                           op0=Alu.mult, op1=Alu.add)
            p2ab = sp.tile([B, 1], fp)
            nc.vector.tensor_scalar_mul(p2ab[:], ab[:], 2.0)
            ncm = sp.tile([B, 1], fp)
            nc.vector.tensor_scalar_add(ncm[:], b_[:], -1.0)
            ncp = sp.tile([B, 1], fp)
            nc.vector.tensor_scalar(ncp[:], b_[:], -1.0, -1.0, Alu.mult, Alu.add)

            # ================= class shard stats (gpsimd + scalar) ========
            kapc = sp.tile([CLOC, D], fp)
            nc.vector.tensor_scalar_max(kapc[:], wk, 0.1)
            msq = scrp.tile([CLOC, D], fp, tag="scCD")
            ssm = sp.tile([CLOC, 1], fp)
            nc.scalar.activation(msq[:], wmu, Act.Square, accum_out=ssm[:])
            lnsm = scrp.tile([CLOC, 1], fp, tag="scC")
            nc.scalar.activation(lnsm[:], ssm[:], Act.Ln)
            rnm = sp.tile([CLOC, 1], fp)
            nc.scalar.activation(rnm[:], lnsm[:], Act.Exp, scale=-0.5)
            scm = sp.tile([CLOC, D], fp)
            nc.vector.tensor_mul(scm[:], wmu, kapc[:])
            nc.vector.tensor_scalar_mul(scm[:], scm[:], rnm[:])
            csq = scrp.tile([CLOC, D], fp, tag="scCD")
            ssc = sp.tile([CLOC, 1], fp)
            nc.scalar.activation(csq[:], scm[:], Act.Square, accum_out=ssc[:])
            Ppb = sp.tile([CLOC, D], bf)
            nc.vector.tensor_mul(Ppb[:], kapc[:], scm[:])
            Qqb = sp.tile([CLOC, D], bf)
            nc.vector.tensor_mul(Qqb[:], kapc[:], kapc[:])
            lkt = scrp.tile([CLOC, D], fp, tag="scCD")
            slk = sp.tile([CLOC, 1], fp)
            nc.scalar.activation(lkt[:], kapc[:], Act.Ln, accum_out=slk[:])
            G = sp.tile([CLOC, 1], fp)
            nc.gpsimd.tensor_scalar_add(G[:], ssc[:], 63.0 * 63.0)
            lnG = sp.tile([CLOC, 1], fp)
            nc.scalar.activation(lnG[:], G[:], Act.Ln)
            eta = sp.tile([CLOC, 1], fp)
            nc.scalar.activation(eta[:], lnG[:], Act.Exp, scale=0.5)
            etap = scrp.tile([CLOC, 1], fp, tag="scC")
            nc.gpsimd.tensor_scalar_add(etap[:], eta[:], 63.0)
            l63 = scrp.tile([CLOC, 1], fp, tag="scC")
            nc.scalar.activation(l63[:], etap[:], Act.Ln)
            lnssc = scrp.tile([CLOC, 1], fp, tag="scC")
            nc.scalar.activation(lnssc[:], ssc[:], Act.Ln)
            c1 = scrp.tile([CLOC, 1], fp, tag="scC")
            nc.gpsimd.tensor_scalar_mul(c1[:], l63[:], 63.0)
            nc.gpsimd.tensor_sub(c1[:], c1[:], eta[:])
            c2 = scrp.tile([CLOC, 1], fp, tag="scC")
            nc.gpsimd.tensor_scalar_mul(c2[:], lnssc[:], -0.5)
            nc.gpsimd.tensor_add(c2[:], c2[:], slk[:])
            nc.gpsimd.tensor_add(c1[:], c1[:], c2[:])
            cst = sp.tile([CLOC, 1], fp)
            nc.gpsimd.tensor_scalar_mul(cst[:], lnG[:], 0.25)
            nc.gpsimd.tensor_add(cst[:], cst[:], c1[:])
            nc.gpsimd.tensor_scalar_add(cst[:], cst[:], -63.5 * LN2PI)

            # class transposes + const broadcast
            identB = geom[:, G_ID:G_ID + 128]
            ps = pp.tile([128, CLOC], bf, tag="a")
            nc.tensor.transpose(ps[:], Ppb[:], identB[0:CLOC, 0:CLOC])
            PpTs = sp.tile([128, CLOC], bf)
            nc.scalar.copy(PpTs[:], ps[:])
            ps = pp.tile([128, CLOC], bf, tag="a")
            nc.tensor.transpose(ps[:], Qqb[:], identB[0:CLOC, 0:CLOC])
            QqTs = sp.tile([128, CLOC], bf)
            nc.vector.tensor_copy(QqTs[:], ps[:])
            ps = pp.tile([1, CLOC], fp, tag="a")
            nc.tensor.transpose(ps[:], cst[:], c32[0:CLOC, C_ID:C_ID + CLOC])
            cstTs = sp.tile([1, CLOC], fp)
            nc.scalar.copy(cstTs[:], ps[:])
            ps_cb = pp.tile([128, CLOC], fp, tag="cb", bufs=1)
            nc.tensor.matmul(ps_cb[:], c32[0:1, C_ONER:C_ONER + 128], cstTs[:],
                             start=True, stop=True)

            # ================= accept [B, SK], split V/G ===================
            x_ = sp.tile([B, SK], fp)
            den = sp.tile([B, SK], fp)
            u_ = sp.tile([B, SK], fp)
            acc = sp.tile([B, SK], fp)
            A = sp.tile([B, SK], fp)
            HF = SK // 2
            nc.vector.tensor_scalar_mul(x_[:], eps, ncm[:])
            nc.vector.tensor_scalar_add(den[:], x_[:], 1.0)
            nc.vector.tensor_scalar(u_[:], x_[:], 1.0 / 3.0, -0.5, Alu.mult, Alu.add)
            nc.vector.tensor_mul(u_[:], u_[:], x_[:])
            nc.vector.scalar_tensor_tensor(acc[:], u_[:], 1.0, x_[:],
                                           op0=Alu.add, op1=Alu.mult)
            nc.vector.tensor_scalar(acc[:], acc[:], -M1, E[:], Alu.mult, Alu.add)
            nc.vector.tensor_sub(acc[:], acc[:], logu)
            nc.vector.tensor_mul(acc[:], acc[:], den[:])
            nc.vector.tensor_scalar(A[:], acc[:], p2ab[:], None, Alu.is_ge)

            P = sp.tile([B, SK], fp)
            nc.vector.tensor_tensor_scan(P[:], c32[0:B, C_RM:C_RM + SK], A[:],
                                         0.0, op0=Alu.mult, op1=Alu.max)
            Pm1 = sp.tile([B, SK], fp)
            nc.vector.tensor_copy(Pm1[:, 1:SK], P[:, 0:SK - 1])
            Pm1_v = Pm1.rearrange("p (s r) -> p s r", r=K)
            nc.gpsimd.memset(Pm1_v[:, :, 0:1], 0.0)
            first = sp.tile([B, SK], fp)
            nc.vector.tensor_sub(first[:], P[:], Pm1[:])
            prod = sp.tile([B, SK], fp)
            nc.vector.tensor_mul(prod[:], eps, first[:])
            esel = sp.tile([B, S], fp)
            nc.vector.tensor_reduce(esel[:],
                                    prod.rearrange("p (s r) -> p s r", r=K),
                                    axis=mybir.AxisListType.X, op=Alu.add)
            fb = scrp.tile([B, S], fp, tag="sc8")
            nc.vector.scalar_tensor_tensor(fb[:], P[:, K - 1::K], 1.0, eps[:, 0::K],
                                           op0=Alu.subtract, op1=Alu.mult)
            nc.vector.tensor_sub(esel[:], esel[:], fb[:])

            # ================= w, sm -> interleaved pack [B, 16] ===========
            pack = sp.tile([B, 2 * S], bf)
            pk = pack.rearrange("p (s two) -> p s two", two=2)
            n1 = scrp.tile([B, S], fp, tag="sc8")
            nc.vector.tensor_scalar(n1[:], esel[:], ncp[:], 1.0, Alu.mult, Alu.add)
            d1 = scrp.tile([B, S], fp, tag="sc8")
            nc.vector.tensor_scalar(d1[:], esel[:], ncm[:], 1.0, Alu.mult, Alu.add)
            rd1 = scrp.tile([B, S], fp, tag="sc8")
            nc.vector.reciprocal(rd1[:], d1[:])
            w_ = sp.tile([B, S], fp)
            nc.vector.tensor_mul(w_[:], n1[:], rd1[:])
            nc.vector.tensor_copy(pk[:, :, 0:1], w_.rearrange("p (s o) -> p s o", o=1))
            w2_ = scrp.tile([B, S], fp, tag="sc8")
            nc.vector.tensor_mul(w2_[:], w_[:], w_[:])
            cw = scrp.tile([B, S], fp, tag="sc8")
            nc.scalar.activation(cw[:], w2_[:], Act.Relu, bias=1.0, scale=-1.0)
            lncw = scrp.tile([B, S], fp, tag="sc8")
            nc.scalar.activation(lncw[:], cw[:], Act.Ln)
            nc.scalar.activation(pk[:, :, 1:2], lncw[:], Act.Exp, scale=0.5)

            # ================= transposed-domain samples ===================
            # packT [16, B]
            ps_pT = pp.tile([16, B], bf, tag="a")
            nc.tensor.transpose(ps_pT[:], pack[:], identB[0:B, 0:B])
            pTs = sp.tile([16, B], bf)
            nc.scalar.copy(pTs[:], ps_pT[:])
            # block-diagonal [16, SB] = pTs[q, b] * mask[q, s] (broadcast APs)
            bdiag = sp.tile([16, SB], bf)
            bd_v = bdiag.rearrange("p (s b) -> p s b", b=B)
            pT_v = pTs[:].rearrange("p (s b) -> p s b", s=1)
            mk_v = geom[0:16, G_MSK:G_MSK + S].rearrange("p (s b) -> p s b", b=1)
            pT_b, mk_b = bass.broadcast_tensor_aps(pT_v, mk_v)
            nc.vector.tensor_tensor(bd_v[:, :, :], pT_b, mk_b, op=Alu.mult)
            # uhT tiled [128, SB] (independent of pack; PE order puts it first)
            ps_uh = pp.tile([128, SB], fp, tag="big")
            nc.tensor.matmul(ps_uh[:], uhb[:], geom[0:B, G_ID8:G_ID8 + SB],
                             start=True, stop=True)
            uhsb = sp.tile([128, SB], bf)
            nc.scalar.copy(uhsb[:], ps_uh[:])
            # zT = (pick.T @ bdiag) * Vp
            ps_bd = pp.tile([128, SB], fp, tag="big")
            nc.tensor.matmul(ps_bd[:], geom[0:16, G_PICK:G_PICK + 128], bdiag[:],
                             start=True, stop=True)
            zT = sp.tile([128, SB], bf)
            nc.vector.tensor_mul(zT[:], ps_bd[:], geom[:, G_VP:G_VP + SB])
            # prodz = -2 * uhT * zT; dpB[d, sb] = sum_d' prodz[d', sb] via
            # all-ones lhs (one matmul replaces dp-mm + row copy + bcast-mm)
            prodz = sp.tile([128, SB], bf)
            nc.vector.scalar_tensor_tensor(prodz[:], uhsb[:], -2.0, zT[:],
                                           op0=Alu.mult, op1=Alu.mult)
            ps_db = pp.tile([128, SB], fp, tag="big")
            nc.tensor.matmul(ps_db[:], geom[:, G_ONES:G_ONES + 128], prodz[:],
                             start=True, stop=True)
            tmp = sp.tile([128, SB], bf)
            nc.vector.tensor_mul(tmp[:], uhsb[:], ps_db[:])
            smpT = sp.tile([128, SB], bf)
            nc.vector.tensor_add(smpT[:], tmp[:], zT[:])
            sqT = sp.tile([128, SB], bf)
            nc.vector.tensor_mul(sqT[:], smpT[:], smpT[:])

            # ================= main matmuls + fused epilogue ===============
            pna = pp.tile([128, 4 * CLOC], fp, tag="pn", bufs=1)
            pda = pp.tile([128, 4 * CLOC], fp, tag="pd", bufs=1)
            for mc in range(4):
                nc.tensor.matmul(pna[:, mc * CLOC:(mc + 1) * CLOC],
                                 smpT[:, mc * 128:(mc + 1) * 128], PpTs[:],
                                 start=True, stop=True)
                nc.tensor.matmul(pda[:, mc * CLOC:(mc + 1) * CLOC],
                                 sqT[:, mc * 128:(mc + 1) * 128], QqTs[:],
                                 start=True, stop=True)
            lnd = scrp.tile([128, 4 * CLOC], fp, tag="ep")
            nc.scalar.activation(lnd[:], pda[:], Act.Ln)
            rd = scrp.tile([128, 4 * CLOC], fp, tag="ep")
            nc.scalar.activation(rd[:], lnd[:], Act.Exp, scale=-0.5)
            m1 = scrp.tile([128, 4 * CLOC], fp, tag="ep")
            nc.vector.tensor_mul(m1[:], pna[:], rd[:])
            o = scrp.tile([128, 4 * CLOC], fp, tag="out")
            o_v = o.rearrange("p (m c) -> p m c", c=CLOC)
            m1_v = m1.rearrange("p (m c) -> p m c", c=CLOC)
            cb_v = ps_cb[:].rearrange("p (m c) -> p m c", m=1)
            cb_b, m1_b = bass.broadcast_tensor_aps(cb_v, m1_v)
            nc.vector.tensor_tensor(o_v[:, :, :], m1_b, cb_b, op=Alu.add)
            nc.sync.dma_start(d_out.rearrange("(m p) c -> p m c", m=4), o_v[:, :, :])

    with tile.TileContext(nc) as tc:
        _emit(tc)
    nc.finalize()
    # All scalar-engine activations use funcs in natural_log_exp_and_others
    # (exp, ln, relu, copy, identity, square).  The auto-inserter picks
    # first-match tables and thrashes between exp_and_others and natural_log
    # (1283ns per load); rewrite to the shared table and drop redundant loads.
    from concourse.hw_specs import get_activation_tables
    tabs = list(get_activation_tables(nc.m.arch).items())
    lnexp = next(i for i, (n, fs) in enumerate(tabs)
                 if n == "natural_log_exp_and_others")
    seen = False
    for blk in nc.m.functions[0].blocks:
        keep = []
        for ins in blk.instructions:
            if isinstance(ins, mybir.InstLoadActFuncSet):
                if seen:
                    continue
                ins.act_func_set_id = lnexp
                seen = True
            keep.append(ins)
        blk.instructions[:] = keep
    return nc


def _get_nc():
    if "nc" not in _cache:
        _cache["nc"] = build_nc()
    return _cache["nc"]


def make_in_maps(inputs):
    eps_b, logu_b, geom_bf, rmask = _host_constants()
    f32 = np.float32

    # mlp packs (input-dependent bf16 hi/lo)
    mlp0 = np.zeros((128, M0_COLS), np.float32)
    mlp1 = np.zeros((128, M1_COLS), np.float32)
    W0T = np.asarray(inputs["W0"], f32).T          # [D, H]
    W1T = np.asarray(inputs["W1"], f32).T          # [H, H]
    fT = np.asarray(inputs["features"], f32).T     # [D, B]
    W2 = np.asarray(inputs["W2"], f32)             # [1, H]
    h, l = _hilo(W0T)
    mlp0[:, M_W0H:M_W0H + H] = h.astype(f32)
    mlp0[:, M_W0L:M_W0L + H] = l.astype(f32)
    h, l = _hilo(fT)
    mlp0[:, M_FH:M_FH + B] = h.astype(f32)
    mlp0[:, M_FL:M_FL + B] = l.astype(f32)
    h, l = _hilo(W2.reshape(2, 128).T)             # col j = W2[0, j*128:(j+1)*128]
    mlp0[:, M_W2H:M_W2H + 2] = h.astype(f32)
    mlp0[:, M_W2L:M_W2L + 2] = l.astype(f32)
    h, l = _hilo(W1T)
    for i in range(2):
        for j in range(2):
            blk = slice(M_W1H + (i * 2 + j) * 128, M_W1H + (i * 2 + j + 1) * 128)
            mlp1[:, blk] = h[i * 128:(i + 1) * 128, j * 128:(j + 1) * 128].astype(f32)
            blk = slice(M_W1L + (i * 2 + j) * 128, M_W1L + (i * 2 + j + 1) * 128)
            mlp1[:, blk] = l[i * 128:(i + 1) * 128, j * 128:(j + 1) * 128].astype(f32)
    mlp0_bf = _bf16(mlp0)
    mlp1_bf = _bf16(mlp1)

    c32 = np.zeros((128, C_COLS), f32)
    for j in range(2):
        c32[:, C_B0 + j] = np.asarray(inputs["b0"], f32)[j * 128:(j + 1) * 128]
        c32[:, C_B1 + j] = np.asarray(inputs["b1"], f32)[j * 128:(j + 1) * 128]
    c32[0:CLOC, C_ID:C_ID + CLOC] = np.eye(CLOC)
    c32[0:1, C_ONER:C_ONER + 128] = 1.0
    c32[0:B, C_RM:C_RM + SK] = rmask

    in64 = np.zeros((B, I_COLS), f32)
    in64[:, I_EPS:I_EPS + SK] = eps_b
    in64[:, I_LOGU:I_LOGU + SK] = logu_b
    in64[:, I_FEAT:I_FEAT + D] = np.asarray(inputs["features"], f32)
    in64[:, I_B2] = float(np.asarray(inputs["b2"], f32)[0])

    com = {
        "mlp0": mlp0_bf,
        "mlp1": mlp1_bf,
        "geom": geom_bf,
        "c32": np.ascontiguousarray(c32),
        "in64": np.ascontiguousarray(in64),
    }
    wmu = np.asarray(inputs["W_mu"], f32)
    wkap = np.asarray(inputs["W_kappa"], f32)
    in_maps = []
    for i in range(NCORES):
        m = dict(com)
        wc = np.empty((CLOC, 2 * D), f32)
        wc[:, 0:D] = wmu[i * CLOC:(i + 1) * CLOC]
        wc[:, D:2 * D] = wkap[i * CLOC:(i + 1) * CLOC]
        m["wcls"] = wc
        in_maps.append(m)
    return in_maps


def kernel(**inputs):
    from concourse.bass_utils import run_bass_kernel_spmd

    nc = _get_nc()
    in_maps = make_in_maps(inputs)
    res = run_bass_kernel_spmd(nc, in_maps, list(range(NCORES)))
    parts = [res.results[i]["out"].reshape(S, B, CLOC) for i in range(NCORES)]
    return np.ascontiguousarray(np.concatenate(parts, axis=2).astype(np.float32))


# revision 16
# speedup vs baseline: 1.8043x; 1.0617x over previous
"""Trainium2 Bass kernel for the NonIsotropic vMF head (v2).

Contract: kernel(**inputs) takes FULL unsharded inputs (as produced by
setup_inputs()) and returns the FULL [S=8, B=64, C=1000] float32 output.

v2 strategy (vs v1 baseline):
  * 5 consolidated input DMAs issued in parallel from 5 engine queues
    (v1: ~24 serialized on Sync).
  * W0/W1/features transposed HOST-side; MLP runs as bf16 hi/lo split
    (3 bf16 matmuls emulate an fp32 matmul to ~1e-6 rel) -- kappa keeps
    fp32-grade accuracy for the rejection-accept margins.
  * Single activation table: only {exp, ln, relu, copy, identity, square}
    are used on the scalar engine; sqrt/rsqrt computed as exp(+-0.5*ln x).
  * Accept test reformulated reciprocal-free:
        margin >= 0  <=>  (E - 127*log1p(x) - logu) * denom >= 2ab.
  * Sample assembly + Householder reflection done directly in the
    transposed [D, S*B] domain with 5 small bf16 matmuls (block-diag
    broadcast trick) instead of 8 fp32 PE transposes + per-sample loops.
  * Class matmuls (num/den) in bf16 (output |val| >= 30, tol 2e-2).

RNG draws (beta/uniform/normal, key 42) are input-independent and
generated host-side exactly as the reference does, shipped as constants.
"""

import numpy as np

S, B, D, K, C, H = 8, 64, 128, 32, 1000, 256
NCORES = 8
CLOC = C // NCORES            # 125 classes per core
SB = S * B                    # 512
SK = S * K                    # 256
M1 = float(D - 1)             # 127.0
LN127 = float(np.log(M1))
LN2PI = float(np.log(2.0 * np.pi))

# ---- geom pack (bf16) [128, G_COLS]: host constants ----
G_ID = 0                      # ident        [0:128, 0:128]
G_VP = 128                    # Vp           [0:128, 128:640] row0=ones, 1..=vT
G_PICK = 640                  # pick         [0:16, 640:768]
G_MSK = 768                   # blockdiag mask [0:16, 768:776]
G_COLS = 776

# ---- mlp packs (bf16): input-derived hi/lo weights ----
# mlp0 [128, 644]: W0 + features + W2 (everything h0/h2 needs)
M_W0H, M_W0L = 0, 256
M_FH, M_FL = 512, 576
M_W2H, M_W2L = 640, 642
M0_COLS = 644
# mlp1 [128, 1024]: four 128-col W1 blocks, order (i,j), hi then lo
M_W1H, M_W1L = 0, 512
M1_COLS = 1024

# ---- c32 pack (fp32) [128, C_COLS] ----
C_B0 = 0                      # [0:128, 0:2]
C_B1 = 2                      # [0:128, 2:4]
C_COLS = 4

# ---- in64 pack (fp32) [64, I_COLS] ----
I_EPS, I_LOGU, I_FEAT, I_B2 = 0, 256, 512, 640
I_COLS = 641

_cache = {}


def _bf16(x):
    import ml_dtypes
    return np.ascontiguousarray(np.asarray(x, np.float32).astype(ml_dtypes.bfloat16))


def _hilo(x):
    import ml_dtypes
    x = np.asarray(x, np.float32)
    hi = x.astype(ml_dtypes.bfloat16)
    lo = (x - hi.astype(np.float32)).astype(ml_dtypes.bfloat16)
    return np.ascontiguousarray(hi), np.ascontiguousarray(lo)


def _host_constants():
    """RNG draws of the reference sampler (key 42) + geometry constants."""
    if "rng" in _cache:
        return _cache["rng"]
    import jax
    import jax.numpy as jnp
    import ml_dtypes

    cpu = jax.devices("cpu")[0]
    with jax.default_device(cpu):
        key = jax.random.key(42)
        k_eps, k_u, k_v = jax.random.split(key, 3)
        alpha = M1 / 2.0
        eps = np.asarray(jax.random.beta(k_eps, alpha, alpha, (K, S, B)), np.float32)
        u = jax.random.uniform(k_u, (K, S, B), jnp.float32, minval=1e-7, maxval=1.0)
        logu = np.asarray(jnp.log(u), np.float32)
        vraw = jax.random.normal(k_v, (S, B, D - 1), jnp.float32)
        vn = np.asarray(
            vraw / jnp.maximum(jnp.linalg.norm(vraw, axis=-1, keepdims=True), 1e-12),
            np.float32,
        )
    eps_b = np.ascontiguousarray(np.transpose(eps, (2, 1, 0)).reshape(B, SK))
    logu_b = np.ascontiguousarray(np.transpose(logu, (2, 1, 0)).reshape(B, SK))

    geom = np.zeros((128, G_COLS), np.float32)
    geom[:, G_ID:G_ID + 128] = np.eye(128)
    geom[0, G_VP:G_VP + SB] = 1.0
    geom[1:128, G_VP:G_VP + SB] = np.transpose(vn, (2, 0, 1)).reshape(D - 1, SB)
    # pick[2s, 0] = 1 (w row), pick[2s+1, 1:] = 1 (sm rows)
    geom[0:16:2, G_PICK] = 1.0
    geom[1:16:2, G_PICK + 1:G_PICK + 128] = 1.0
    for q in range(16):
        geom[q, G_MSK + q // 2] = 1.0
    geom_bf = np.ascontiguousarray(geom.astype(ml_dtypes.bfloat16))

    _cache["rng"] = (eps_b, logu_b, geom_bf)
    return _cache["rng"]


def build_nc():
    import concourse.bass as bass
    import concourse.mybir as mybir
    from concourse import bacc, tile

    fp = mybir.dt.float32
    bf = mybir.dt.bfloat16
    Alu = mybir.AluOpType
    Act = mybir.ActivationFunctionType

    nc = bacc.Bacc(None)

    d_mlp0 = nc.declare_dram_parameter("mlp0", [128, M0_COLS], bf, isOutput=False)
    d_mlp1 = nc.declare_dram_parameter("mlp1", [128, M1_COLS], bf, isOutput=False)
    d_geom = nc.declare_dram_parameter("geom", [128, G_COLS], bf, isOutput=False)
    d_c32 = nc.declare_dram_parameter("c32", [128, C_COLS], fp, isOutput=False)
    d_in64 = nc.declare_dram_parameter("in64", [B, I_COLS], fp, isOutput=False)
    d_wcls = nc.declare_dram_parameter("wcls", [CLOC, 2 * D], fp, isOutput=False)
    d_out = nc.declare_dram_parameter("out", [SB, CLOC], fp, isOutput=True)

    def _emit(tc):
        with (
            tc.tile_pool(name="w", bufs=1) as wp,
            tc.tile_pool(name="s", bufs=1) as sp,
            tc.tile_pool(name="scr", bufs=4) as scrp,
            tc.tile_pool(name="pp", bufs=2, space="PSUM") as pp,
        ):
            # ================= loads (parallel issue, 5 queues) =============
            mlp0 = wp.tile([128, M0_COLS], bf)
            nc.sync.dma_start(mlp0[:], d_mlp0[:])
            mlp1 = wp.tile([128, M1_COLS], bf)
            nc.sync.dma_start(mlp1[:], d_mlp1[:])
            wcl = wp.tile([CLOC, 2 * D], fp)
            nc.sync.dma_start(wcl[:], d_wcls[:])
            in64 = wp.tile([B, I_COLS], fp)
            nc.scalar.dma_start(in64[:], d_in64[:])
            c32 = wp.tile([128, C_COLS], fp)
            nc.scalar.dma_start(c32[:], d_c32[:])
            geom = wp.tile([128, G_COLS], bf)
            nc.gpsimd.dma_start(geom[:], d_geom[:])

            rmask = wp.tile([B, SK], fp)
            nc.gpsimd.memset(rmask[:], 1.0)
            rm_v = rmask.rearrange("p (s r) -> p s r", r=K)
            nc.gpsimd.memset(rm_v[:, :, 0:1], 0.0)
            m2ones = wp.tile([128, 128], bf)
            nc.gpsimd.memset(m2ones[:], -2.0)
            eps = in64[:, I_EPS:I_EPS + SK]
            logu = in64[:, I_LOGU:I_LOGU + SK]
            feat = in64[:, I_FEAT:I_FEAT + D]
            b2r = in64[:, I_B2:I_B2 + 1]
            wmu = wcl[:, 0:D]
            wk = wcl[:, D:2 * D]

            # ================= MLP (bf16 hi/lo) ============================
            fTh = mlp0[:, M_FH:M_FH + B]
            fTl = mlp0[:, M_FL:M_FL + B]
            h0r = [sp.tile([128, B], fp, name=f"h0r{j}") for j in range(2)]
            h0h = [sp.tile([128, B], bf, name=f"h0h{j}") for j in range(2)]
            h0l = [sp.tile([128, B], bf, name=f"h0l{j}") for j in range(2)]
            for j in range(2):
                w0h = mlp0[:, M_W0H + j * 128:M_W0H + (j + 1) * 128]
                w0l = mlp0[:, M_W0L + j * 128:M_W0L + (j + 1) * 128]
                pm = pp.tile([128, B], fp, tag="a")
                nc.tensor.matmul(pm[:], w0h, fTh, start=True, stop=False)
                nc.tensor.matmul(pm[:], w0h, fTl, start=False, stop=False)
                nc.tensor.matmul(pm[:], w0l, fTh, start=False, stop=True)
                nc.scalar.activation(h0h[j][:], pm[:], Act.Relu,
                                     bias=c32[:, C_B0 + j:C_B0 + j + 1], scale=1.0)
                nc.scalar.activation(h0r[j][:], pm[:], Act.Relu,
                                     bias=c32[:, C_B0 + j:C_B0 + j + 1], scale=1.0)
                nc.vector.scalar_tensor_tensor(h0l[j][:], h0h[j][:], -1.0,
                                               h0r[j][:], op0=Alu.mult, op1=Alu.add)

            # ---- uh chain part 1 (needs only feat; fills scalar idle) ----
            fsq = scrp.tile([B, D], fp, tag="scBD")
            ssf = sp.tile([B, 1], fp)
            nc.scalar.activation(fsq[:], feat, Act.Square, accum_out=ssf[:])
            lnf = scrp.tile([B, 1], fp, tag="sc")
            nc.scalar.activation(lnf[:], ssf[:], Act.Ln)
            rnf = sp.tile([B, 1], fp)
            nc.scalar.activation(rnf[:], lnf[:], Act.Exp, scale=-0.5)
            em = sp.tile([B, D], fp)
            nc.vector.tensor_scalar(em[:], feat, rnf[:], -1.0, Alu.mult, Alu.mult)
            nc.vector.tensor_scalar_add(em[:, 0:1], em[:, 0:1], 1.0)

            h1r = [sp.tile([128, B], fp, name=f"h1r{j}") for j in range(2)]
            h1h = [sp.tile([128, B], bf, name=f"h1h{j}") for j in range(2)]
            h1l = [sp.tile([128, B], bf, name=f"h1l{j}") for j in range(2)]
            w1b = lambda hl, i, j: mlp1[:, hl + (i * 2 + j) * 128:hl + (i * 2 + j + 1) * 128]
            pmj = [pp.tile([128, B], fp, tag="a", name=f"pmh1{j}") for j in range(2)]
            for j in range(2):
                nc.tensor.matmul(pmj[j][:], w1b(M_W1H, 0, j), h0h[0][:], start=True, stop=False)
                nc.tensor.matmul(pmj[j][:], w1b(M_W1L, 0, j), h0h[0][:], start=False, stop=False)
                nc.tensor.matmul(pmj[j][:], w1b(M_W1H, 1, j), h0h[1][:], start=False, stop=False)
                nc.tensor.matmul(pmj[j][:], w1b(M_W1L, 1, j), h0h[1][:], start=False, stop=False)
                nc.tensor.matmul(pmj[j][:], w1b(M_W1H, 0, j), h0l[0][:], start=False, stop=False)
                nc.tensor.matmul(pmj[j][:], w1b(M_W1H, 1, j), h0l[1][:], start=False, stop=True)
            for j in range(2):
                nc.scalar.activation(h1h[j][:], pmj[j][:], Act.Relu,
                                     bias=c32[:, C_B1 + j:C_B1 + j + 1], scale=1.0)
                nc.scalar.activation(h1r[j][:], pmj[j][:], Act.Relu,
                                     bias=c32[:, C_B1 + j:C_B1 + j + 1], scale=1.0)
                nc.vector.scalar_tensor_tensor(h1l[j][:], h1h[j][:], -1.0,
                                               h1r[j][:], op0=Alu.mult, op1=Alu.add)

            # ---- uh chain part 2 ----
            esq = scrp.tile([B, D], fp, tag="scBD")
            sse = sp.tile([B, 1], fp)
            nc.scalar.activation(esq[:], em[:], Act.Square, accum_out=sse[:])
            lne = scrp.tile([B, 1], fp, tag="sc")
            nc.scalar.activation(lne[:], sse[:], Act.Ln)
            rne = sp.tile([B, 1], fp)
            nc.scalar.activation(rne[:], lne[:], Act.Exp, scale=-0.5)
            uhb = sp.tile([B, D], bf)
            nc.vector.tensor_scalar_mul(uhb[:], em[:], rne[:])

            # ---- h2 + softplus ----
            pm2 = pp.tile([B, 1], fp, tag="a")
            for j in range(2):
                w2h = mlp0[:, M_W2H + j:M_W2H + j + 1]
                w2l = mlp0[:, M_W2L + j:M_W2L + j + 1]
                nc.tensor.matmul(pm2[:], h1h[j][:], w2h, start=(j == 0), stop=False)
                nc.tensor.matmul(pm2[:], h1h[j][:], w2l, start=False, stop=False)
                nc.tensor.matmul(pm2[:], h1l[j][:], w2h, start=False, stop=(j == 1))
            eh2 = sp.tile([B, 1], fp)
            nc.scalar.activation(eh2[:], pm2[:], Act.Exp, bias=b2r, scale=1.0)
            kapb = sp.tile([B, 1], fp)
            nc.scalar.activation(kapb[:], eh2[:], Act.Ln, bias=1.0, scale=1.0)
            nc.vector.tensor_scalar_add(kapb[:], kapb[:], 1e-6)

            # ================= sampler scalars [B,1] =======================
            k2 = scrp.tile([B, 1], fp, tag="sc")
            nc.vector.tensor_mul(k2[:], kapb[:], kapb[:])
            nc.vector.tensor_scalar(k2[:], k2[:], 4.0, M1 * M1, Alu.mult, Alu.add)
            lnk2 = scrp.tile([B, 1], fp, tag="sc")
            nc.scalar.activation(lnk2[:], k2[:], Act.Ln)
            sq = sp.tile([B, 1], fp)
            nc.scalar.activation(sq[:], lnk2[:], Act.Exp, scale=0.5)
            b_ = sp.tile([B, 1], fp)
            nc.vector.scalar_tensor_tensor(b_[:], kapb[:], -2.0, sq[:],
                                           op0=Alu.mult, op1=Alu.add)
            nc.vector.tensor_scalar_mul(b_[:], b_[:], 1.0 / M1)
            a_ = sp.tile([B, 1], fp)
            nc.vector.scalar_tensor_tensor(a_[:], kapb[:], 2.0, sq[:],
                                           op0=Alu.mult, op1=Alu.add)
            nc.vector.tensor_scalar(a_[:], a_[:], M1, 0.25, Alu.add, Alu.mult)
            ab = sp.tile([B, 1], fp)
            nc.vector.tensor_mul(ab[:], a_[:], b_[:])
            opb = scrp.tile([B, 1], fp, tag="sc")
            nc.vector.tensor_scalar_add(opb[:], b_[:], 1.0)
            r1pb = scrp.tile([B, 1], fp, tag="sc")
            nc.vector.reciprocal(r1pb[:], opb[:])
            d_ = sp.tile([B, 1], fp)
            nc.vector.scalar_tensor_tensor(d_[:], ab[:], 4.0, r1pb[:],
                                           op0=Alu.mult, op1=Alu.mult)
            nc.vector.tensor_scalar_add(d_[:], d_[:], -M1 * LN127)
            l2ab = scrp.tile([B, 1], fp, tag="sc")
            nc.scalar.activation(l2ab[:], ab[:], Act.Ln, scale=2.0)
            E = sp.tile([B, 1], fp)
            nc.vector.scalar_tensor_tensor(E[:], l2ab[:], M1, d_[:],
                                           op0=Alu.mult, op1=Alu.add)
            p2ab = sp.tile([B, 1], fp)
            nc.vector.tensor_scalar_mul(p2ab[:], ab[:], 2.0)
            ncm = sp.tile([B, 1], fp)
            nc.vector.tensor_scalar_add(ncm[:], b_[:], -1.0)
            ncp = sp.tile([B, 1], fp)
            nc.vector.tensor_scalar(ncp[:], b_[:], -1.0, -1.0, Alu.mult, Alu.add)

            # ================= class shard stats (gpsimd + scalar) ========
            kapc = sp.tile([CLOC, D], fp)
            nc.vector.tensor_scalar_max(kapc[:], wk, 0.1)
            msq = scrp.tile([CLOC, D], fp, tag="scCD")
            ssm = sp.tile([CLOC, 1], fp)
            nc.scalar.activation(msq[:], wmu, Act.Square, accum_out=ssm[:])
            lnsm = scrp.tile([CLOC, 1], fp, tag="scC")
            nc.scalar.activation(lnsm[:], ssm[:], Act.Ln)
            rnm = sp.tile([CLOC, 1], fp)
            nc.scalar.activation(rnm[:], lnsm[:], Act.Exp, scale=-0.5)
            scm = sp.tile([CLOC, D], fp)
            nc.vector.tensor_mul(scm[:], wmu, kapc[:])
            nc.vector.tensor_scalar_mul(scm[:], scm[:], rnm[:])
            csq = scrp.tile([CLOC, D], fp, tag="scCD")
            ssc = sp.tile([CLOC, 1], fp)
            nc.scalar.activation(csq[:], scm[:], Act.Square, accum_out=ssc[:])
            Ppb = sp.tile([CLOC, D], bf)
            nc.vector.tensor_mul(Ppb[:], kapc[:], scm[:])
            Qqb = sp.tile([CLOC, D], bf)
            nc.vector.tensor_mul(Qqb[:], kapc[:], kapc[:])
            lkt = scrp.tile([CLOC, D], fp, tag="scCD")
            slk = sp.tile([CLOC, 1], fp)
            nc.scalar.activation(lkt[:], kapc[:], Act.Ln, accum_out=slk[:])
            G = sp.tile([CLOC, 1], fp)
            nc.gpsimd.tensor_scalar_add(G[:], ssc[:], 63.0 * 63.0)
            lnG = sp.tile([CLOC, 1], fp)
            nc.scalar.activation(lnG[:], G[:], Act.Ln)
            eta = sp.tile([CLOC, 1], fp)
            nc.scalar.activation(eta[:], lnG[:], Act.Exp, scale=0.5)
            etap = scrp.tile([CLOC, 1], fp, tag="scC")
            nc.gpsimd.tensor_scalar_add(etap[:], eta[:], 63.0)
            l63 = scrp.tile([CLOC, 1], fp, tag="scC")
            nc.scalar.activation(l63[:], etap[:], Act.Ln)
            lnssc = scrp.tile([CLOC, 1], fp, tag="scC")
            nc.scalar.activation(lnssc[:], ssc[:], Act.Ln)
            c1 = scrp.tile([CLOC, 1], fp, tag="scC")
            nc.gpsimd.tensor_scalar_mul(c1[:], l63[:], 63.0)
            nc.gpsimd.tensor_sub(c1[:], c1[:], eta[:])
            c2 = scrp.tile([CLOC, 1], fp, tag="scC")
            nc.gpsimd.tensor_scalar_mul(c2[:], lnssc[:], -0.5)
            nc.gpsimd.tensor_add(c2[:], c2[:], slk[:])
            nc.gpsimd.tensor_add(c1[:], c1[:], c2[:])
            cst = sp.tile([CLOC, 1], fp)
            nc.gpsimd.tensor_scalar_mul(cst[:], lnG[:], 0.25)
            nc.gpsimd.tensor_add(cst[:], cst[:], c1[:])
            nc.gpsimd.tensor_scalar_add(cst[:], cst[:], -63.5 * LN2PI)

            # class transposes + const broadcast
            identB = geom[:, G_ID:G_ID + 128]
            ps = pp.tile([128, CLOC], bf, tag="a")
            nc.tensor.transpose(ps[:], Ppb[:], identB[0:CLOC, 0:CLOC])
            PpTs = sp.tile([128, CLOC], bf)
            nc.scalar.copy(PpTs[:], ps[:])
            ps = pp.tile([128, CLOC], bf, tag="a")
            nc.tensor.transpose(ps[:], Qqb[:], identB[0:CLOC, 0:CLOC])
            QqTs = sp.tile([128, CLOC], bf)
            nc.vector.tensor_copy(QqTs[:], ps[:])
            cstb = sp.tile([CLOC, 1], bf)
            nc.gpsimd.tensor_copy(cstb[:], cst[:])
            ps = pp.tile([1, CLOC], bf, tag="a")
            nc.tensor.transpose(ps[:], cstb[:], identB[0:CLOC, 0:CLOC])
            cstTs = sp.tile([1, CLOC], bf)
            nc.scalar.copy(cstTs[:], ps[:])
            ps_cb = pp.tile([128, CLOC], fp, tag="cb", bufs=1)
            nc.tensor.matmul(ps_cb[:], geom[0:1, G_VP:G_VP + 128], cstTs[:],
                             start=True, stop=True)

            # ================= accept [B, SK], split V/G ===================
            x_ = sp.tile([B, SK], fp)
            den = sp.tile([B, SK], fp)
            u_ = sp.tile([B, SK], fp)
            acc = sp.tile([B, SK], fp)
            A = sp.tile([B, SK], fp)
            HF = SK // 2
            nc.vector.tensor_scalar_mul(x_[:], eps, ncm[:])
            nc.vector.tensor_scalar_add(den[:], x_[:], 1.0)
            nc.vector.tensor_scalar(u_[:], x_[:], 1.0 / 3.0, -0.5, Alu.mult, Alu.add)
            nc.vector.tensor_mul(u_[:], u_[:], x_[:])
            nc.vector.scalar_tensor_tensor(acc[:], u_[:], 1.0, x_[:],
                                           op0=Alu.add, op1=Alu.mult)
            nc.vector.tensor_scalar(acc[:], acc[:], -M1, E[:], Alu.mult, Alu.add)
            nc.vector.tensor_sub(acc[:], acc[:], logu)
            nc.vector.tensor_mul(acc[:], acc[:], den[:])
            nc.vector.tensor_scalar(A[:], acc[:], p2ab[:], None, Alu.is_ge)

            P = sp.tile([B, SK], fp)
            nc.vector.tensor_tensor_scan(P[:], rmask[:], A[:],
                                         0.0, op0=Alu.mult, op1=Alu.max)
            Pm1 = sp.tile([B, SK], fp)
            nc.vector.tensor_copy(Pm1[:, 1:SK], P[:, 0:SK - 1])
            Pm1_v = Pm1.rearrange("p (s r) -> p s r", r=K)
            nc.gpsimd.memset(Pm1_v[:, :, 0:1], 0.0)
            first = sp.tile([B, SK], fp)
            nc.vector.tensor_sub(first[:], P[:], Pm1[:])
            prod = sp.tile([B, SK], fp)
            nc.vector.tensor_mul(prod[:], eps, first[:])
            esel = sp.tile([B, S], fp)
            nc.vector.tensor_reduce(esel[:],
                                    prod.rearrange("p (s r) -> p s r", r=K),
                                    axis=mybir.AxisListType.X, op=Alu.add)
            fb = scrp.tile([B, S], fp, tag="sc8")
            nc.vector.scalar_tensor_tensor(fb[:], P[:, K - 1::K], 1.0, eps[:, 0::K],
                                           op0=Alu.subtract, op1=Alu.mult)
            nc.vector.tensor_sub(esel[:], esel[:], fb[:])

            # ================= w, sm -> interleaved pack [B, 16] ===========
            pack = sp.tile([B, 2 * S], bf)
            pk = pack.rearrange("p (s two) -> p s two", two=2)
            n1 = scrp.tile([B, S], fp, tag="sc8")
            nc.vector.tensor_scalar(n1[:], esel[:], ncp[:], 1.0, Alu.mult, Alu.add)
            d1 = scrp.tile([B, S], fp, tag="sc8")
            nc.vector.tensor_scalar(d1[:], esel[:], ncm[:], 1.0, Alu.mult, Alu.add)
            rd1 = scrp.tile([B, S], fp, tag="sc8")
            nc.vector.reciprocal(rd1[:], d1[:])
            w_ = sp.tile([B, S], fp)
            nc.vector.tensor_mul(w_[:], n1[:], rd1[:])
            nc.vector.tensor_copy(pk[:, :, 0:1], w_.rearrange("p (s o) -> p s o", o=1))
            w2_ = scrp.tile([B, S], fp, tag="sc8")
            nc.vector.tensor_mul(w2_[:], w_[:], w_[:])
            cw = scrp.tile([B, S], fp, tag="sc8")
            nc.scalar.activation(cw[:], w2_[:], Act.Relu, bias=1.0, scale=-1.0)
            lncw = scrp.tile([B, S], fp, tag="sc8")
            nc.scalar.activation(lncw[:], cw[:], Act.Ln)
            nc.scalar.activation(pk[:, :, 1:2], lncw[:], Act.Exp, scale=0.5)

            # ================= transposed-domain samples ===================
            # packT [16, B]
            ps_pT = pp.tile([16, B], bf, tag="a")
            nc.tensor.transpose(ps_pT[:], pack[:], identB[0:B, 0:B])
            pTs = sp.tile([16, B], bf)
            nc.scalar.copy(pTs[:], ps_pT[:])
            # block-diagonal [16, SB] = pTs[q, b] * mask[q, s] (broadcast APs)
            bdiag = sp.tile([16, SB], bf)
            bd_v = bdiag.rearrange("p (s b) -> p s b", b=B)
            pT_v = pTs[:].rearrange("p (s b) -> p s b", s=1)
            mk_v = geom[0:16, G_MSK:G_MSK + S].rearrange("p (s b) -> p s b", b=1)
            pT_b, mk_b = bass.broadcast_tensor_aps(pT_v, mk_v)
            nc.vector.tensor_tensor(bd_v[:, :, :], pT_b, mk_b, op=Alu.mult)
            # uhT [128, B] via transpose, tiled x8 with a stride-0 broadcast copy
            ps_uh = pp.tile([128, B], bf, tag="a")
            nc.tensor.transpose(ps_uh[:], uhb[:], identB[0:B, 0:B])
            uhT_s = sp.tile([128, B], bf)
            nc.scalar.copy(uhT_s[:], ps_uh[:])
            uhsb = sp.tile([128, SB], bf)
            uh_v = uhsb.rearrange("p (s b) -> p s b", b=B)
            us_v = uhT_s[:].rearrange("p (s b) -> p s b", s=1)
            us_b, uh_b = bass.broadcast_tensor_aps(us_v, uh_v)
            nc.vector.tensor_copy(uh_v[:, :, :], us_b)
            # Vu = Vp * uhT (off critical path; feeds prodz)
            Vu = sp.tile([128, SB], bf)
            nc.vector.tensor_mul(Vu[:], geom[:, G_VP:G_VP + SB], uhsb[:])
            # zT = (pick.T @ bdiag) * Vp
            ps_bd = pp.tile([128, SB], fp, tag="big")
            nc.tensor.matmul(ps_bd[:], geom[0:16, G_PICK:G_PICK + 128], bdiag[:],
                             start=True, stop=True)
            # prodz = uhT*(bd*Vp) = bd*Vu first (feeds the dp reduction);
            # zT computed after, overlapping the dpB matmul.
            prodz = sp.tile([128, SB], bf)
            nc.vector.tensor_mul(prodz[:], ps_bd[:], Vu[:])
            ps_db = pp.tile([128, SB], fp, tag="big")
            nc.tensor.matmul(ps_db[:], m2ones[:], prodz[:],
                             start=True, stop=True)
            zT = sp.tile([128, SB], bf)
            nc.vector.tensor_mul(zT[:], ps_bd[:], geom[:, G_VP:G_VP + SB])
            tmp = sp.tile([128, SB], bf)
            nc.vector.tensor_mul(tmp[:], uhsb[:], ps_db[:])
            smpT = sp.tile([128, SB], bf)
            nc.vector.tensor_add(smpT[:], tmp[:], zT[:])
            sqT = sp.tile([128, SB], bf)
            nc.scalar.activation(sqT[:], smpT[:], Act.Square)

            # ================= main matmuls + fused epilogue ===============
            pna = pp.tile([128, 4 * CLOC], fp, tag="pn", bufs=1)
            pda = pp.tile([128, 4 * CLOC], fp, tag="pd", bufs=1)
            for mc in range(4):
                nc.tensor.matmul(pna[:, mc * CLOC:(mc + 1) * CLOC],
                                 smpT[:, mc * 128:(mc + 1) * 128], PpTs[:],
                                 start=True, stop=True)
                nc.tensor.matmul(pda[:, mc * CLOC:(mc + 1) * CLOC],
                                 sqT[:, mc * 128:(mc + 1) * 128], QqTs[:],
                                 start=True, stop=True)
            lnd = scrp.tile([128, 4 * CLOC], fp, tag="ep")
            nc.scalar.activation(lnd[:], pda[:], Act.Ln)
            rd = scrp.tile([128, 4 * CLOC], fp, tag="ep")
            nc.scalar.activation(rd[:], lnd[:], Act.Exp, scale=-0.5)
            m1 = scrp.tile([128, 4 * CLOC], fp, tag="ep")
            nc.vector.tensor_mul(m1[:], pna[:], rd[:])
            o = scrp.tile([128, 4 * CLOC], fp, tag="out")
            o_v = o.rearrange("p (m c) -> p m c", c=CLOC)
            m1_v = m1.rearrange("p (m c) -> p m c", c=CLOC)
            cb_v = ps_cb[:].rearrange("p (m c) -> p m c", m=1)
            cb_b, m1_b = bass.broadcast_tensor_aps(cb_v, m1_v)
            nc.vector.tensor_tensor(o_v[:, :, :], m1_b, cb_b, op=Alu.add)
            nc.sync.dma_start(d_out.rearrange("(m p) c -> p m c", m=4), o_v[:, :, :])

    with tile.TileContext(nc) as tc:
        _emit(tc)
    nc.finalize()
    # All scalar-engine activations use funcs in natural_log_exp_and_others
    # (exp, ln, relu, copy, identity, square).  The auto-inserter picks
    # first-match tables and thrashes between exp_and_others and natural_log
    # (1283ns per load); rewrite to the shared table and drop redundant loads.
    from concourse.hw_specs import get_activation_tables
    tabs = list(get_activation_tables(nc.m.arch).items())
    lnexp = next(i for i, (n, fs) in enumerate(tabs)
                 if n == "natural_log_exp_and_others")
    seen = False
    for blk in nc.m.functions[0].blocks:
        keep = []
        for ins in blk.instructions:
            if isinstance(ins, mybir.InstLoadActFuncSet):
                if seen:
                    continue
                ins.act_func_set_id = lnexp
                seen = True
            keep.append(ins)
        blk.instructions[:] = keep
    return nc


def _get_nc():
    if "nc" not in _cache:
        _cache["nc"] = build_nc()
    return _cache["nc"]


def make_in_maps(inputs):
    eps_b, logu_b, geom_bf = _host_constants()
    f32 = np.float32

    # mlp packs (input-dependent bf16 hi/lo)
    mlp0 = np.zeros((128, M0_COLS), np.float32)
    mlp1 = np.zeros((128, M1_COLS), np.float32)
    W0T = np.asarray(inputs["W0"], f32).T          # [D, H]
    W1T = np.asarray(inputs["W1"], f32).T          # [H, H]
    fT = np.asarray(inputs["features"], f32).T     # [D, B]
    W2 = np.asarray(inputs["W2"], f32)             # [1, H]
    h, l = _hilo(W0T)
    mlp0[:, M_W0H:M_W0H + H] = h.astype(f32)
    mlp0[:, M_W0L:M_W0L + H] = l.astype(f32)
    h, l = _hilo(fT)
    mlp0[:, M_FH:M_FH + B] = h.astype(f32)
    mlp0[:, M_FL:M_FL + B] = l.astype(f32)
    h, l = _hilo(W2.reshape(2, 128).T)             # col j = W2[0, j*128:(j+1)*128]
    mlp0[:, M_W2H:M_W2H + 2] = h.astype(f32)
    mlp0[:, M_W2L:M_W2L + 2] = l.astype(f32)
    h, l = _hilo(W1T)
    for i in range(2):
        for j in range(2):
            blk = slice(M_W1H + (i * 2 + j) * 128, M_W1H + (i * 2 + j + 1) * 128)
            mlp1[:, blk] = h[i * 128:(i + 1) * 128, j * 128:(j + 1) * 128].astype(f32)
            blk = slice(M_W1L + (i * 2 + j) * 128, M_W1L + (i * 2 + j + 1) * 128)
            mlp1[:, blk] = l[i * 128:(i + 1) * 128, j * 128:(j + 1) * 128].astype(f32)
    mlp0_bf = _bf16(mlp0)
    mlp1_bf = _bf16(mlp1)

    c32 = np.zeros((128, C_COLS), f32)
    for j in range(2):
        c32[:, C_B0 + j] = np.asarray(inputs["b0"], f32)[j * 128:(j + 1) * 128]
        c32[:, C_B1 + j] = np.asarray(inputs["b1"], f32)[j * 128:(j + 1) * 128]

    in64 = np.zeros((B, I_COLS), f32)
    in64[:, I_EPS:I_EPS + SK] = eps_b
    in64[:, I_LOGU:I_LOGU + SK] = logu_b
    in64[:, I_FEAT:I_FEAT + D] = np.asarray(inputs["features"], f32)
    in64[:, I_B2] = float(np.asarray(inputs["b2"], f32)[0])

    com = {
        "mlp0": mlp0_bf,
        "mlp1": mlp1_bf,
        "geom": geom_bf,
        "c32": np.ascontiguousarray(c32),
        "in64": np.ascontiguousarray(in64),
    }
    wmu = np.asarray(inputs["W_mu"], f32)
    wkap = np.asarray(inputs["W_kappa"], f32)
    in_maps = []
    for i in range(NCORES):
        m = dict(com)
        wc = np.empty((CLOC, 2 * D), f32)
        wc[:, 0:D] = wmu[i * CLOC:(i + 1) * CLOC]
        wc[:, D:2 * D] = wkap[i * CLOC:(i + 1) * CLOC]
        m["wcls"] = wc
        in_maps.append(m)
    return in_maps


def kernel(**inputs):
    from concourse.bass_utils import run_bass_kernel_spmd

    nc = _get_nc()
    in_maps = make_in_maps(inputs)
    res = run_bass_kernel_spmd(nc, in_maps, list(range(NCORES)))
    parts = [res.results[i]["out"].reshape(S, B, CLOC) for i in range(NCORES)]
    return np.ascontiguousarray(np.concatenate(parts, axis=2).astype(np.float32))


# revision 17
# speedup vs baseline: 1.8101x; 1.0032x over previous
"""Trainium2 Bass kernel for the NonIsotropic vMF head (v2).

Contract: kernel(**inputs) takes FULL unsharded inputs (as produced by
setup_inputs()) and returns the FULL [S=8, B=64, C=1000] float32 output.

v2 strategy (vs v1 baseline):
  * 5 consolidated input DMAs issued in parallel from 5 engine queues
    (v1: ~24 serialized on Sync).
  * W0/W1/features transposed HOST-side; MLP runs as bf16 hi/lo split
    (3 bf16 matmuls emulate an fp32 matmul to ~1e-6 rel) -- kappa keeps
    fp32-grade accuracy for the rejection-accept margins.
  * Single activation table: only {exp, ln, relu, copy, identity, square}
    are used on the scalar engine; sqrt/rsqrt computed as exp(+-0.5*ln x).
  * Accept test reformulated reciprocal-free:
        margin >= 0  <=>  (E - 127*log1p(x) - logu) * denom >= 2ab.
  * Sample assembly + Householder reflection done directly in the
    transposed [D, S*B] domain with 5 small bf16 matmuls (block-diag
    broadcast trick) instead of 8 fp32 PE transposes + per-sample loops.
  * Class matmuls (num/den) in bf16 (output |val| >= 30, tol 2e-2).

RNG draws (beta/uniform/normal, key 42) are input-independent and
generated host-side exactly as the reference does, shipped as constants.
"""

import numpy as np

S, B, D, K, C, H = 8, 64, 128, 32, 1000, 256
NCORES = 8
CLOC = C // NCORES            # 125 classes per core
SB = S * B                    # 512
SK = S * K                    # 256
M1 = float(D - 1)             # 127.0
LN127 = float(np.log(M1))
LN2PI = float(np.log(2.0 * np.pi))

# ---- geom pack (bf16) [128, G_COLS]: host constants ----
G_ID = 0                      # ident        [0:128, 0:128]
G_VP = 128                    # Vp           [0:128, 128:640] row0=ones, 1..=vT
G_PICK = 640                  # pick         [0:16, 640:768]
G_MSK = 768                   # blockdiag mask [0:16, 768:776]
G_COLS = 776

# ---- mlp packs (bf16): input-derived hi/lo weights ----
# mlp0 [128, 644]: W0 + features + W2 (everything h0/h2 needs)
M_W0H, M_W0L = 0, 256
M_FH, M_FL = 512, 576
M_W2H, M_W2L = 640, 642
M0_COLS = 644
# mlp1 [128, 1024]: four 128-col W1 blocks, order (i,j), hi then lo
M_W1H, M_W1L = 0, 512
M1_COLS = 1024

# ---- c32 pack (fp32) [128, C_COLS] ----
C_B0 = 0                      # [0:128, 0:2]
C_B1 = 2                      # [0:128, 2:4]
C_COLS = 4

# ---- in64 pack (fp32) [64, I_COLS] ----
I_EPS, I_LOGU, I_FEAT, I_B2 = 0, 256, 512, 640
I_COLS = 641

_cache = {}


def _bf16(x):
    import ml_dtypes
    return np.ascontiguousarray(np.asarray(x, np.float32).astype(ml_dtypes.bfloat16))


def _hilo(x):
    import ml_dtypes
    x = np.asarray(x, np.float32)
    hi = x.astype(ml_dtypes.bfloat16)
    lo = (x - hi.astype(np.float32)).astype(ml_dtypes.bfloat16)
    return np.ascontiguousarray(hi), np.ascontiguousarray(lo)


def _host_constants():
    """RNG draws of the reference sampler (key 42) + geometry constants."""
    if "rng" in _cache:
        return _cache["rng"]
    import jax
    import jax.numpy as jnp
    import ml_dtypes

    cpu = jax.devices("cpu")[0]
    with jax.default_device(cpu):
        key = jax.random.key(42)
        k_eps, k_u, k_v = jax.random.split(key, 3)
        alpha = M1 / 2.0
        eps = np.asarray(jax.random.beta(k_eps, alpha, alpha, (K, S, B)), np.float32)
        u = jax.random.uniform(k_u, (K, S, B), jnp.float32, minval=1e-7, maxval=1.0)
        logu = np.asarray(jnp.log(u), np.float32)
        vraw = jax.random.normal(k_v, (S, B, D - 1), jnp.float32)
        vn = np.asarray(
            vraw / jnp.maximum(jnp.linalg.norm(vraw, axis=-1, keepdims=True), 1e-12),
            np.float32,
        )
    eps_b = np.ascontiguousarray(np.transpose(eps, (2, 1, 0)).reshape(B, SK))
    logu_b = np.ascontiguousarray(np.transpose(logu, (2, 1, 0)).reshape(B, SK))

    geom = np.zeros((128, G_COLS), np.float32)
    geom[:, G_ID:G_ID + 128] = np.eye(128)
    geom[0, G_VP:G_VP + SB] = 1.0
    geom[1:128, G_VP:G_VP + SB] = np.transpose(vn, (2, 0, 1)).reshape(D - 1, SB)
    # pick[2s, 0] = 1 (w row), pick[2s+1, 1:] = 1 (sm rows)
    geom[0:16:2, G_PICK] = 1.0
    geom[1:16:2, G_PICK + 1:G_PICK + 128] = 1.0
    for q in range(16):
        geom[q, G_MSK + q // 2] = 1.0
    geom_bf = np.ascontiguousarray(geom.astype(ml_dtypes.bfloat16))

    _cache["rng"] = (eps_b, logu_b, geom_bf)
    return _cache["rng"]


def build_nc():
    import concourse.bass as bass
    import concourse.mybir as mybir
    from concourse import bacc, tile

    fp = mybir.dt.float32
    bf = mybir.dt.bfloat16
    Alu = mybir.AluOpType
    Act = mybir.ActivationFunctionType

    nc = bacc.Bacc(None)

    d_mlp0 = nc.declare_dram_parameter("mlp0", [128, M0_COLS], bf, isOutput=False)
    d_mlp1 = nc.declare_dram_parameter("mlp1", [128, M1_COLS], bf, isOutput=False)
    d_geom = nc.declare_dram_parameter("geom", [128, G_COLS], bf, isOutput=False)
    d_c32 = nc.declare_dram_parameter("c32", [128, C_COLS], fp, isOutput=False)
    d_in64 = nc.declare_dram_parameter("in64", [B, I_COLS], fp, isOutput=False)
    d_wcls = nc.declare_dram_parameter("wcls", [CLOC, 2 * D], fp, isOutput=False)
    d_out = nc.declare_dram_parameter("out", [SB, CLOC], fp, isOutput=True)

    def _emit(tc):
        with (
            tc.tile_pool(name="w", bufs=1) as wp,
            tc.tile_pool(name="s", bufs=1) as sp,
            tc.tile_pool(name="scr", bufs=4) as scrp,
            tc.tile_pool(name="pp", bufs=2, space="PSUM") as pp,
        ):
            # ================= loads (parallel issue, 5 queues) =============
            mlp0 = wp.tile([128, M0_COLS], bf)
            nc.sync.dma_start(mlp0[:], d_mlp0[:])
            mlp1 = wp.tile([128, M1_COLS], bf)
            nc.sync.dma_start(mlp1[:], d_mlp1[:])
            wcl = wp.tile([CLOC, 2 * D], fp)
            nc.sync.dma_start(wcl[:], d_wcls[:])
            in64 = wp.tile([B, I_COLS], fp)
            nc.scalar.dma_start(in64[:], d_in64[:])
            c32 = wp.tile([128, C_COLS], fp)
            nc.scalar.dma_start(c32[:], d_c32[:])
            geom = wp.tile([128, G_COLS], bf)
            nc.gpsimd.dma_start(geom[:], d_geom[:])

            rmask = wp.tile([B, SK], fp)
            nc.gpsimd.memset(rmask[:], 1.0)
            rm_v = rmask.rearrange("p (s r) -> p s r", r=K)
            nc.gpsimd.memset(rm_v[:, :, 0:1], 0.0)
            m2ones = wp.tile([128, 128], bf)
            nc.gpsimd.memset(m2ones[:], -2.0)
            eps = in64[:, I_EPS:I_EPS + SK]
            logu = in64[:, I_LOGU:I_LOGU + SK]
            feat = in64[:, I_FEAT:I_FEAT + D]
            b2r = in64[:, I_B2:I_B2 + 1]
            wmu = wcl[:, 0:D]
            wk = wcl[:, D:2 * D]

            # ================= MLP (bf16 hi/lo) ============================
            fTh = mlp0[:, M_FH:M_FH + B]
            fTl = mlp0[:, M_FL:M_FL + B]
            h0r = [sp.tile([128, B], fp, name=f"h0r{j}") for j in range(2)]
            h0h = [sp.tile([128, B], bf, name=f"h0h{j}") for j in range(2)]
            h0l = [sp.tile([128, B], bf, name=f"h0l{j}") for j in range(2)]
            for j in range(2):
                w0h = mlp0[:, M_W0H + j * 128:M_W0H + (j + 1) * 128]
                w0l = mlp0[:, M_W0L + j * 128:M_W0L + (j + 1) * 128]
                pm = pp.tile([128, B], fp, tag="a")
                nc.tensor.matmul(pm[:], w0h, fTh, start=True, stop=False)
                nc.tensor.matmul(pm[:], w0h, fTl, start=False, stop=False)
                nc.tensor.matmul(pm[:], w0l, fTh, start=False, stop=True)
                nc.scalar.activation(h0h[j][:], pm[:], Act.Relu,
                                     bias=c32[:, C_B0 + j:C_B0 + j + 1], scale=1.0)
                nc.scalar.activation(h0r[j][:], pm[:], Act.Relu,
                                     bias=c32[:, C_B0 + j:C_B0 + j + 1], scale=1.0)
                nc.vector.scalar_tensor_tensor(h0l[j][:], h0h[j][:], -1.0,
                                               h0r[j][:], op0=Alu.mult, op1=Alu.add)

            # ---- uh chain part 1 (needs only feat; fills scalar idle) ----
            fsq = scrp.tile([B, D], fp, tag="scBD")
            ssf = sp.tile([B, 1], fp)
            nc.scalar.activation(fsq[:], feat, Act.Square, accum_out=ssf[:])
            lnf = scrp.tile([B, 1], fp, tag="sc")
            nc.scalar.activation(lnf[:], ssf[:], Act.Ln)
            rnf = sp.tile([B, 1], fp)
            nc.scalar.activation(rnf[:], lnf[:], Act.Exp, scale=-0.5)
            em = sp.tile([B, D], fp)
            nc.vector.tensor_scalar(em[:], feat, rnf[:], -1.0, Alu.mult, Alu.mult)
            nc.vector.tensor_scalar_add(em[:, 0:1], em[:, 0:1], 1.0)

            h1r = [sp.tile([128, B], fp, name=f"h1r{j}") for j in range(2)]
            h1h = [sp.tile([128, B], bf, name=f"h1h{j}") for j in range(2)]
            h1l = [sp.tile([128, B], bf, name=f"h1l{j}") for j in range(2)]
            w1b = lambda hl, i, j: mlp1[:, hl + (i * 2 + j) * 128:hl + (i * 2 + j + 1) * 128]
            pmj = [pp.tile([128, B], fp, tag="a", name=f"pmh1{j}") for j in range(2)]
            for j in range(2):
                nc.tensor.matmul(pmj[j][:], w1b(M_W1H, 0, j), h0h[0][:], start=True, stop=False)
                nc.tensor.matmul(pmj[j][:], w1b(M_W1L, 0, j), h0h[0][:], start=False, stop=False)
                nc.tensor.matmul(pmj[j][:], w1b(M_W1H, 1, j), h0h[1][:], start=False, stop=False)
                nc.tensor.matmul(pmj[j][:], w1b(M_W1L, 1, j), h0h[1][:], start=False, stop=False)
                nc.tensor.matmul(pmj[j][:], w1b(M_W1H, 0, j), h0l[0][:], start=False, stop=False)
                nc.tensor.matmul(pmj[j][:], w1b(M_W1H, 1, j), h0l[1][:], start=False, stop=True)
            for j in range(2):
                nc.scalar.activation(h1h[j][:], pmj[j][:], Act.Relu,
                                     bias=c32[:, C_B1 + j:C_B1 + j + 1], scale=1.0)
                nc.scalar.activation(h1r[j][:], pmj[j][:], Act.Relu,
                                     bias=c32[:, C_B1 + j:C_B1 + j + 1], scale=1.0)
                nc.vector.scalar_tensor_tensor(h1l[j][:], h1h[j][:], -1.0,
                                               h1r[j][:], op0=Alu.mult, op1=Alu.add)

            # ---- uh chain part 2 ----
            esq = scrp.tile([B, D], fp, tag="scBD")
            sse = sp.tile([B, 1], fp)
            nc.scalar.activation(esq[:], em[:], Act.Square, accum_out=sse[:])
            lne = scrp.tile([B, 1], fp, tag="sc")
            nc.scalar.activation(lne[:], sse[:], Act.Ln)
            rne = sp.tile([B, 1], fp)
            nc.scalar.activation(rne[:], lne[:], Act.Exp, scale=-0.5)
            uhb = sp.tile([B, D], bf)
            nc.vector.tensor_scalar_mul(uhb[:], em[:], rne[:])

            # ---- h2 + softplus ----
            pm2 = pp.tile([B, 1], fp, tag="a")
            for j in range(2):
                w2h = mlp0[:, M_W2H + j:M_W2H + j + 1]
                w2l = mlp0[:, M_W2L + j:M_W2L + j + 1]
                nc.tensor.matmul(pm2[:], h1h[j][:], w2h, start=(j == 0), stop=False)
                nc.tensor.matmul(pm2[:], h1h[j][:], w2l, start=False, stop=False)
                nc.tensor.matmul(pm2[:], h1l[j][:], w2h, start=False, stop=(j == 1))
            eh2 = sp.tile([B, 1], fp)
            nc.scalar.activation(eh2[:], pm2[:], Act.Exp, bias=b2r, scale=1.0)
            kapb = sp.tile([B, 1], fp)
            nc.scalar.activation(kapb[:], eh2[:], Act.Ln, bias=1.0, scale=1.0)
            nc.vector.tensor_scalar_add(kapb[:], kapb[:], 1e-6)

            # ================= sampler scalars [B,1] =======================
            k2 = scrp.tile([B, 1], fp, tag="sc")
            nc.vector.tensor_mul(k2[:], kapb[:], kapb[:])
            nc.vector.tensor_scalar(k2[:], k2[:], 4.0, M1 * M1, Alu.mult, Alu.add)
            lnk2 = scrp.tile([B, 1], fp, tag="sc")
            nc.scalar.activation(lnk2[:], k2[:], Act.Ln)
            sq = sp.tile([B, 1], fp)
            nc.scalar.activation(sq[:], lnk2[:], Act.Exp, scale=0.5)
            b_ = sp.tile([B, 1], fp)
            nc.vector.scalar_tensor_tensor(b_[:], kapb[:], -2.0, sq[:],
                                           op0=Alu.mult, op1=Alu.add)
            nc.vector.tensor_scalar_mul(b_[:], b_[:], 1.0 / M1)
            a_ = sp.tile([B, 1], fp)
            nc.vector.scalar_tensor_tensor(a_[:], kapb[:], 2.0, sq[:],
                                           op0=Alu.mult, op1=Alu.add)
            nc.vector.tensor_scalar(a_[:], a_[:], M1, 0.25, Alu.add, Alu.mult)
            ab = sp.tile([B, 1], fp)
            nc.vector.tensor_mul(ab[:], a_[:], b_[:])
            opb = scrp.tile([B, 1], fp, tag="sc")
            nc.vector.tensor_scalar_add(opb[:], b_[:], 1.0)
            r1pb = scrp.tile([B, 1], fp, tag="sc")
            nc.vector.reciprocal(r1pb[:], opb[:])
            d_ = sp.tile([B, 1], fp)
            nc.vector.scalar_tensor_tensor(d_[:], ab[:], 4.0, r1pb[:],
                                           op0=Alu.mult, op1=Alu.mult)
            nc.vector.tensor_scalar_add(d_[:], d_[:], -M1 * LN127)
            l2ab = scrp.tile([B, 1], fp, tag="sc")
            nc.scalar.activation(l2ab[:], ab[:], Act.Ln, scale=2.0)
            E = sp.tile([B, 1], fp)
            nc.vector.scalar_tensor_tensor(E[:], l2ab[:], M1, d_[:],
                                           op0=Alu.mult, op1=Alu.add)
            p2ab = sp.tile([B, 1], fp)
            nc.vector.tensor_scalar_mul(p2ab[:], ab[:], 2.0)
            ncm = sp.tile([B, 1], fp)
            nc.vector.tensor_scalar_add(ncm[:], b_[:], -1.0)
            ncp = sp.tile([B, 1], fp)
            nc.vector.tensor_scalar(ncp[:], b_[:], -1.0, -1.0, Alu.mult, Alu.add)

            # ================= class shard stats (gpsimd + scalar) ========
            kapc = sp.tile([CLOC, D], fp)
            nc.gpsimd.tensor_scalar_max(kapc[:], wk, 0.1)
            msq = scrp.tile([CLOC, D], fp, tag="scCD")
            ssm = sp.tile([CLOC, 1], fp)
            nc.scalar.activation(msq[:], wmu, Act.Square, accum_out=ssm[:])
            lnsm = scrp.tile([CLOC, 1], fp, tag="scC")
            nc.scalar.activation(lnsm[:], ssm[:], Act.Ln)
            rnm = sp.tile([CLOC, 1], fp)
            nc.scalar.activation(rnm[:], lnsm[:], Act.Exp, scale=-0.5)
            scm = sp.tile([CLOC, D], fp)
            nc.gpsimd.tensor_mul(scm[:], wmu, kapc[:])
            nc.vector.tensor_scalar_mul(scm[:], scm[:], rnm[:])
            csq = scrp.tile([CLOC, D], fp, tag="scCD")
            ssc = sp.tile([CLOC, 1], fp)
            nc.scalar.activation(csq[:], scm[:], Act.Square, accum_out=ssc[:])
            Ppb = sp.tile([CLOC, D], bf)
            nc.gpsimd.tensor_mul(Ppb[:], kapc[:], scm[:])
            Qqb = sp.tile([CLOC, D], bf)
            nc.gpsimd.tensor_mul(Qqb[:], kapc[:], kapc[:])
            lkt = scrp.tile([CLOC, D], fp, tag="scCD")
            slk = sp.tile([CLOC, 1], fp)
            nc.scalar.activation(lkt[:], kapc[:], Act.Ln, accum_out=slk[:])
            G = sp.tile([CLOC, 1], fp)
            nc.gpsimd.tensor_scalar_add(G[:], ssc[:], 63.0 * 63.0)
            lnG = sp.tile([CLOC, 1], fp)
            nc.scalar.activation(lnG[:], G[:], Act.Ln)
            eta = sp.tile([CLOC, 1], fp)
            nc.scalar.activation(eta[:], lnG[:], Act.Exp, scale=0.5)
            etap = scrp.tile([CLOC, 1], fp, tag="scC")
            nc.gpsimd.tensor_scalar_add(etap[:], eta[:], 63.0)
            l63 = scrp.tile([CLOC, 1], fp, tag="scC")
            nc.scalar.activation(l63[:], etap[:], Act.Ln)
            lnssc = scrp.tile([CLOC, 1], fp, tag="scC")
            nc.scalar.activation(lnssc[:], ssc[:], Act.Ln)
            c1 = scrp.tile([CLOC, 1], fp, tag="scC")
            nc.gpsimd.tensor_scalar_mul(c1[:], l63[:], 63.0)
            nc.gpsimd.tensor_sub(c1[:], c1[:], eta[:])
            c2 = scrp.tile([CLOC, 1], fp, tag="scC")
            nc.gpsimd.tensor_scalar_mul(c2[:], lnssc[:], -0.5)
            nc.gpsimd.tensor_add(c2[:], c2[:], slk[:])
            nc.gpsimd.tensor_add(c1[:], c1[:], c2[:])
            cst = sp.tile([CLOC, 1], fp)
            nc.gpsimd.tensor_scalar_mul(cst[:], lnG[:], 0.25)
            nc.gpsimd.tensor_add(cst[:], cst[:], c1[:])
            nc.gpsimd.tensor_scalar_add(cst[:], cst[:], -63.5 * LN2PI)

            # class transposes + const broadcast
            identB = geom[:, G_ID:G_ID + 128]
            ps = pp.tile([128, CLOC], bf, tag="a")
            nc.tensor.transpose(ps[:], Ppb[:], identB[0:CLOC, 0:CLOC])
            PpTs = sp.tile([128, CLOC], bf)
            nc.scalar.copy(PpTs[:], ps[:])
            ps = pp.tile([128, CLOC], bf, tag="a")
            nc.tensor.transpose(ps[:], Qqb[:], identB[0:CLOC, 0:CLOC])
            QqTs = sp.tile([128, CLOC], bf)
            nc.vector.tensor_copy(QqTs[:], ps[:])
            cstb = sp.tile([CLOC, 1], bf)
            nc.gpsimd.tensor_copy(cstb[:], cst[:])
            ps = pp.tile([1, CLOC], bf, tag="a")
            nc.tensor.transpose(ps[:], cstb[:], identB[0:CLOC, 0:CLOC])
            cstTs = sp.tile([1, CLOC], bf)
            nc.scalar.copy(cstTs[:], ps[:])
            ps_cb = pp.tile([128, CLOC], fp, tag="cb", bufs=1)
            nc.tensor.matmul(ps_cb[:], geom[0:1, G_VP:G_VP + 128], cstTs[:],
                             start=True, stop=True)

            # ================= accept [B, SK], split V/G ===================
            x_ = sp.tile([B, SK], fp)
            den = sp.tile([B, SK], fp)
            u_ = sp.tile([B, SK], fp)
            acc = sp.tile([B, SK], fp)
            A = sp.tile([B, SK], fp)
            HF = SK // 2
            nc.vector.tensor_scalar_mul(x_[:], eps, ncm[:])
            nc.vector.tensor_scalar_add(den[:], x_[:], 1.0)
            nc.vector.tensor_scalar(u_[:], x_[:], 1.0 / 3.0, -0.5, Alu.mult, Alu.add)
            nc.vector.tensor_mul(u_[:], u_[:], x_[:])
            nc.vector.scalar_tensor_tensor(acc[:], u_[:], 1.0, x_[:],
                                           op0=Alu.add, op1=Alu.mult)
            nc.vector.tensor_scalar(acc[:], acc[:], -M1, E[:], Alu.mult, Alu.add)
            nc.vector.tensor_sub(acc[:], acc[:], logu)
            nc.vector.tensor_mul(acc[:], acc[:], den[:])
            nc.vector.tensor_scalar(A[:], acc[:], p2ab[:], None, Alu.is_ge)

            P = sp.tile([B, SK], fp)
            nc.vector.tensor_tensor_scan(P[:], rmask[:], A[:],
                                         0.0, op0=Alu.mult, op1=Alu.max)
            Pm1 = sp.tile([B, SK], fp)
            nc.vector.tensor_copy(Pm1[:, 1:SK], P[:, 0:SK - 1])
            Pm1_v = Pm1.rearrange("p (s r) -> p s r", r=K)
            nc.gpsimd.memset(Pm1_v[:, :, 0:1], 0.0)
            first = sp.tile([B, SK], fp)
            nc.vector.tensor_sub(first[:], P[:], Pm1[:])
            prod = sp.tile([B, SK], fp)
            nc.vector.tensor_mul(prod[:], eps, first[:])
            esel = sp.tile([B, S], fp)
            nc.vector.tensor_reduce(esel[:],
                                    prod.rearrange("p (s r) -> p s r", r=K),
                                    axis=mybir.AxisListType.X, op=Alu.add)
            fb = scrp.tile([B, S], fp, tag="sc8")
            nc.vector.scalar_tensor_tensor(fb[:], P[:, K - 1::K], 1.0, eps[:, 0::K],
                                           op0=Alu.subtract, op1=Alu.mult)
            nc.vector.tensor_sub(esel[:], esel[:], fb[:])

            # ================= w, sm -> interleaved pack [B, 16] ===========
            pack = sp.tile([B, 2 * S], bf)
            pk = pack.rearrange("p (s two) -> p s two", two=2)
            n1 = scrp.tile([B, S], fp, tag="sc8")
            nc.vector.tensor_scalar(n1[:], esel[:], ncp[:], 1.0, Alu.mult, Alu.add)
            d1 = scrp.tile([B, S], fp, tag="sc8")
            nc.vector.tensor_scalar(d1[:], esel[:], ncm[:], 1.0, Alu.mult, Alu.add)
            rd1 = scrp.tile([B, S], fp, tag="sc8")
            nc.vector.reciprocal(rd1[:], d1[:])
            w_ = sp.tile([B, S], fp)
            nc.vector.tensor_mul(w_[:], n1[:], rd1[:])
            nc.vector.tensor_copy(pk[:, :, 0:1], w_.rearrange("p (s o) -> p s o", o=1))
            w2_ = scrp.tile([B, S], fp, tag="sc8")
            nc.vector.tensor_mul(w2_[:], w_[:], w_[:])
            cw = scrp.tile([B, S], fp, tag="sc8")
            nc.scalar.activation(cw[:], w2_[:], Act.Relu, bias=1.0, scale=-1.0)
            lncw = scrp.tile([B, S], fp, tag="sc8")
            nc.scalar.activation(lncw[:], cw[:], Act.Ln)
            nc.scalar.activation(pk[:, :, 1:2], lncw[:], Act.Exp, scale=0.5)

            # ================= transposed-domain samples ===================
            # packT [16, B]
            ps_pT = pp.tile([16, B], bf, tag="a")
            nc.tensor.transpose(ps_pT[:], pack[:], identB[0:B, 0:B])
            pTs = sp.tile([16, B], bf)
            nc.scalar.copy(pTs[:], ps_pT[:])
            # block-diagonal [16, SB] = pTs[q, b] * mask[q, s] (broadcast APs)
            bdiag = sp.tile([16, SB], bf)
            bd_v = bdiag.rearrange("p (s b) -> p s b", b=B)
            pT_v = pTs[:].rearrange("p (s b) -> p s b", s=1)
            mk_v = geom[0:16, G_MSK:G_MSK + S].rearrange("p (s b) -> p s b", b=1)
            pT_b, mk_b = bass.broadcast_tensor_aps(pT_v, mk_v)
            nc.vector.tensor_tensor(bd_v[:, :, :], pT_b, mk_b, op=Alu.mult)
            # uhT [128, B] via transpose, tiled x8 with a stride-0 broadcast copy
            ps_uh = pp.tile([128, B], bf, tag="a")
            nc.tensor.transpose(ps_uh[:], uhb[:], identB[0:B, 0:B])
            uhT_s = sp.tile([128, B], bf)
            nc.scalar.copy(uhT_s[:], ps_uh[:])
            uhsb = sp.tile([128, SB], bf)
            uh_v = uhsb.rearrange("p (s b) -> p s b", b=B)
            us_v = uhT_s[:].rearrange("p (s b) -> p s b", s=1)
            us_b, uh_b = bass.broadcast_tensor_aps(us_v, uh_v)
            nc.vector.tensor_copy(uh_v[:, :, :], us_b)
            # Vu = Vp * uhT (off critical path; feeds prodz)
            Vu = sp.tile([128, SB], bf)
            nc.vector.tensor_mul(Vu[:], geom[:, G_VP:G_VP + SB], uhsb[:])
            # zT = (pick.T @ bdiag) * Vp
            ps_bd = pp.tile([128, SB], fp, tag="big")
            nc.tensor.matmul(ps_bd[:], geom[0:16, G_PICK:G_PICK + 128], bdiag[:],
                             start=True, stop=True)
            # prodz = uhT*(bd*Vp) = bd*Vu first (feeds the dp reduction);
            # zT computed after, overlapping the dpB matmul.
            prodz = sp.tile([128, SB], bf)
            nc.vector.tensor_mul(prodz[:], ps_bd[:], Vu[:])
            ps_db = pp.tile([128, SB], fp, tag="big")
            nc.tensor.matmul(ps_db[:], m2ones[:], prodz[:],
                             start=True, stop=True)
            zT = sp.tile([128, SB], bf)
            nc.vector.tensor_mul(zT[:], ps_bd[:], geom[:, G_VP:G_VP + SB])
            tmp = sp.tile([128, SB], bf)
            nc.vector.tensor_mul(tmp[:], uhsb[:], ps_db[:])
            smpT = sp.tile([128, SB], bf)
            nc.vector.tensor_add(smpT[:], tmp[:], zT[:])
            sqT = sp.tile([128, SB], bf)
            nc.vector.tensor_mul(sqT[:], smpT[:], smpT[:])

            # ================= main matmuls + fused epilogue ===============
            pna = pp.tile([128, 4 * CLOC], fp, tag="pn", bufs=1)
            pda = pp.tile([128, 4 * CLOC], fp, tag="pd", bufs=1)
            for mc in range(4):
                nc.tensor.matmul(pna[:, mc * CLOC:(mc + 1) * CLOC],
                                 smpT[:, mc * 128:(mc + 1) * 128], PpTs[:],
                                 start=True, stop=True)
                nc.tensor.matmul(pda[:, mc * CLOC:(mc + 1) * CLOC],
                                 sqT[:, mc * 128:(mc + 1) * 128], QqTs[:],
                                 start=True, stop=True)
            lnd = scrp.tile([128, 4 * CLOC], fp, tag="ep")
            rd = scrp.tile([128, 4 * CLOC], fp, tag="ep")
            m1 = scrp.tile([128, 4 * CLOC], fp, tag="ep")
            o = scrp.tile([128, 4 * CLOC], fp, tag="out")
            o_v = o.rearrange("p (m c) -> p m c", c=CLOC)
            m1_v = m1.rearrange("p (m c) -> p m c", c=CLOC)
            cb_v = ps_cb[:].rearrange("p (m c) -> p m c", m=1)
            d_out_v = d_out.rearrange("(m p) c -> p m c", m=4)
            HC = 2 * CLOC
            for h in range(2):
                cs = slice(h * HC, (h + 1) * HC)
                nc.scalar.activation(lnd[:, cs], pda[:, cs], Act.Ln)
                nc.scalar.activation(rd[:, cs], lnd[:, cs], Act.Exp, scale=-0.5)
                nc.vector.tensor_mul(m1[:, cs], pna[:, cs], rd[:, cs])
                cb_b, m1_b = bass.broadcast_tensor_aps(cb_v, m1_v[:, 2 * h:2 * h + 2, :])
                nc.vector.tensor_tensor(o_v[:, 2 * h:2 * h + 2, :], m1_b, cb_b,
                                        op=Alu.add)
                eng = nc.sync if h == 0 else nc.gpsimd
                eng.dma_start(d_out_v[:, 2 * h:2 * h + 2, :], o_v[:, 2 * h:2 * h + 2, :])

    with tile.TileContext(nc) as tc:
        _emit(tc)
    nc.finalize()
    # All scalar-engine activations use funcs in natural_log_exp_and_others
    # (exp, ln, relu, copy, identity, square).  The auto-inserter picks
    # first-match tables and thrashes between exp_and_others and natural_log
    # (1283ns per load); rewrite to the shared table and drop redundant loads.
    from concourse.hw_specs import get_activation_tables
    tabs = list(get_activation_tables(nc.m.arch).items())
    lnexp = next(i for i, (n, fs) in enumerate(tabs)
                 if n == "natural_log_exp_and_others")
    seen = False
    for blk in nc.m.functions[0].blocks:
        keep = []
        for ins in blk.instructions:
            if isinstance(ins, mybir.InstLoadActFuncSet):
                if seen:
                    continue
                ins.act_func_set_id = lnexp
                seen = True
            keep.append(ins)
        blk.instructions[:] = keep
    return nc


def _get_nc():
    if "nc" not in _cache:
        _cache["nc"] = build_nc()
    return _cache["nc"]


def make_in_maps(inputs):
    eps_b, logu_b, geom_bf = _host_constants()
    f32 = np.float32

    # mlp packs (input-dependent bf16 hi/lo)
    mlp0 = np.zeros((128, M0_COLS), np.float32)
    mlp1 = np.zeros((128, M1_COLS), np.float32)
    W0T = np.asarray(inputs["W0"], f32).T          # [D, H]
    W1T = np.asarray(inputs["W1"], f32).T          # [H, H]
    fT = np.asarray(inputs["features"], f32).T     # [D, B]
    W2 = np.asarray(inputs["W2"], f32)             # [1, H]
    h, l = _hilo(W0T)
    mlp0[:, M_W0H:M_W0H + H] = h.astype(f32)
    mlp0[:, M_W0L:M_W0L + H] = l.astype(f32)
    h, l = _hilo(fT)
    mlp0[:, M_FH:M_FH + B] = h.astype(f32)
    mlp0[:, M_FL:M_FL + B] = l.astype(f32)
    h, l = _hilo(W2.reshape(2, 128).T)             # col j = W2[0, j*128:(j+1)*128]
    mlp0[:, M_W2H:M_W2H + 2] = h.astype(f32)
    mlp0[:, M_W2L:M_W2L + 2] = l.astype(f32)
    h, l = _hilo(W1T)
    for i in range(2):
        for j in range(2):
            blk = slice(M_W1H + (i * 2 + j) * 128, M_W1H + (i * 2 + j + 1) * 128)
            mlp1[:, blk] = h[i * 128:(i + 1) * 128, j * 128:(j + 1) * 128].astype(f32)
            blk = slice(M_W1L + (i * 2 + j) * 128, M_W1L + (i * 2 + j + 1) * 128)
            mlp1[:, blk] = l[i * 128:(i + 1) * 128, j * 128:(j + 1) * 128].astype(f32)
    mlp0_bf = _bf16(mlp0)
    mlp1_bf = _bf16(mlp1)

    c32 = np.zeros((128, C_COLS), f32)
    for j in range(2):
        c32[:, C_B0 + j] = np.asarray(inputs["b0"], f32)[j * 128:(j + 1) * 128]
        c32[:, C_B1 + j] = np.asarray(inputs["b1"], f32)[j * 128:(j + 1) * 128]

    in64 = np.zeros((B, I_COLS), f32)
    in64[:, I_EPS:I_EPS + SK] = eps_b
    in64[:, I_LOGU:I_LOGU + SK] = logu_b
    in64[:, I_FEAT:I_FEAT + D] = np.asarray(inputs["features"], f32)
    in64[:, I_B2] = float(np.asarray(inputs["b2"], f32)[0])

    com = {
        "mlp0": mlp0_bf,
        "mlp1": mlp1_bf,
        "geom": geom_bf,
        "c32": np.ascontiguousarray(c32),
        "in64": np.ascontiguousarray(in64),
    }
    wmu = np.asarray(inputs["W_mu"], f32)
    wkap = np.asarray(inputs["W_kappa"], f32)
    in_maps = []
    for i in range(NCORES):
        m = dict(com)
        wc = np.empty((CLOC, 2 * D), f32)
        wc[:, 0:D] = wmu[i * CLOC:(i + 1) * CLOC]
        wc[:, D:2 * D] = wkap[i * CLOC:(i + 1) * CLOC]
        m["wcls"] = wc
        in_maps.append(m)
    return in_maps


def kernel(**inputs):
    from concourse.bass_utils import run_bass_kernel_spmd

    nc = _get_nc()
    in_maps = make_in_maps(inputs)
    res = run_bass_kernel_spmd(nc, in_maps, list(range(NCORES)))
    parts = [res.results[i]["out"].reshape(S, B, CLOC) for i in range(NCORES)]
    return np.ascontiguousarray(np.concatenate(parts, axis=2).astype(np.float32))
